# revision 1
# baseline (speedup 1.0000x reference)
"""HGCN decoder on 8 trn2 NeuronCores.

Strategy: nodes are sorted by in-degree, grouped into 128-node tiles, and the
tiles are dealt round-robin across the 8 cores (graph-parallel by destination
node).  Each core:
  - runs the node-wise hyperbolic math (HypLinear / exp / log maps) on its
    4096 nodes, tile by tile, with the per-node scalar chains batched into
    [128, 32] arrays,
  - publishes its tangent-space table shard, AllGathers the full [32768, 64]
    table to DRAM,
  - aggregates messages with `dma_gather` (padded per-tile CSR: tile t gathers
    [128, K_t, 64] source rows in one indirect DMA) followed by a weighted
    strided reduce on the vector engine,
  - finishes with the euclidean readout matmul.
All graph preprocessing (permutation, padded neighbor tables, weight folding
of edge/node masks) happens host-side in numpy; the device only sees dense
tables.
"""

import numpy as np

N = 32768
E = 1015808
D = 64
C = 8          # cores
NL = N // C    # 4096 nodes per core
P = 128        # partitions / tile
T = NL // P    # 32 tiles per core
MAXN = 1.0 - 4e-3   # PROJ_EPS boundary for c=1
EPS = 1e-15
ART_CLIP = 1.0 - 1e-5
MAX_TANH = 15.0


def _build_tables(rows, cols, edge_mask, node_mask):
    """Permute nodes by degree, deal tiles round-robin to cores, and build the
    per-core padded gather tables (int16 indices wrapped the way
    InstDMAGatherAnt wants them) plus matching weight tables."""
    deg = np.bincount(rows, minlength=N)
    order = np.argsort(-deg, kind="stable")
    # global tile j -> core j%C, slot j//C ; permuted position of its p-th node
    perm = np.empty(N, dtype=np.int64)
    j = np.arange(N) // P                     # global tile of sorted rank r
    c = j % C
    t = j // C
    p = np.arange(N) % P
    perm[c * NL + t * P + p] = order          # perm[g] = original node id
    pos = np.empty(N, dtype=np.int64)
    pos[perm] = np.arange(N)

    # gather-table row id for permuted position g=(c,t,p):
    #   AllGather concatenates per-core [P, T*D] blocks, so
    #   row_id = c*NL + p*T + t
    gg = np.arange(N)
    gc, gr = gg // NL, gg % NL
    gt, gp_ = gr // P, gr % P
    rowid = gc * NL + gp_ * T + gt            # [g] -> table row
    dstpos = pos[rows]
    eorder = np.argsort(dstpos, kind="stable")
    src_sorted = rowid[pos[cols[eorder]]]     # gather table rows, 0..N-1
    w_sorted = edge_mask[eorder, 0].astype(np.float64)
    cnts = np.bincount(dstpos, minlength=N)
    offs = np.zeros(N + 1, dtype=np.int64)
    np.cumsum(cnts, out=offs[1:])

    # per-slot K: max count over the 8 cores' tiles in that slot
    cnts_g = cnts.reshape(C, T, P)
    Ks = np.maximum(cnts_g.max(axis=(0, 2)), 1).astype(np.int64)   # [T]

    IDXC = int(8 * Ks.sum())
    WTC = int(Ks.sum())
    idx_dev = np.zeros((C, P, IDXC), np.int16)
    wt_dev = np.zeros((C, P, WTC), np.float32)
    nm = node_mask[:, 0].astype(np.float64)
    ioff = woff = 0
    ar = None
    for t in range(T):
        K = int(Ks[t])
        if ar is None or ar.shape[1] != K:
            ar = np.arange(K)[None, :]
        for cc in range(C):
            base = cc * NL + t * P
            cn = cnts[base:base + P]
            take = offs[base:base + P][:, None] + ar          # [P, K]
            valid = ar < cn[:, None]
            take_c = np.minimum(take, E - 1)
            nb = np.where(valid, src_sorted[take_c], 0)
            wl = np.where(valid, w_sorted[take_c], 0.0)
            wl = wl * nm[perm[base:base + P]][:, None]
            il = nb.T.reshape(-1)                             # i = g*128+p
            ch = il.reshape(8 * K, 16).T                      # [16, 8K]
            idx_dev[cc, :, ioff:ioff + 8 * K] = np.tile(ch, (8, 1)).astype(np.int16)
            wt_dev[cc, :, woff:woff + K] = wl.astype(np.float32)
        ioff += 8 * K
        woff += K
    # pad counts per (core, slot, partition) for the pad-subtract path
    pc_dev = np.zeros((C, 1, T * P), np.float32)
    for t in range(T):
        K = int(Ks[t])
        for cc in range(C):
            base = cc * NL + t * P
            pc_dev[cc, 0, t * P:(t + 1) * P] = K - cnts[base:base + P]
    allones = bool(np.all(edge_mask == 1.0) and np.all(node_mask == 1.0))
    return perm, Ks, idx_dev, wt_dev, IDXC, WTC, pc_dev, allones


def _build_program(Ks, IDXC, WTC, use_wt=True, sim=False):
    import os
    import concourse.bacc as bacc
    import concourse.bass as bass
    import concourse.mybir as mybir
    import concourse.tile as tile
    from concourse import library_config
    from concourse.masks import make_identity

    f32 = mybir.dt.float32
    i16 = mybir.dt.int16
    AF = mybir.ActivationFunctionType
    OP = mybir.AluOpType
    X = mybir.AxisListType.X

    nc = bacc.Bacc("TRN2", target_bir_lowering=False, debug=False,
                   num_devices=1 if sim else C)
    ablate = set(os.environ.get("KABLATE", "").split(",")) if sim else set()

    h_in = nc.dram_tensor("h_in", [P, T * D], f32, kind="ExternalInput")
    idx_in = nc.dram_tensor("idx_in", [P, IDXC], i16, kind="ExternalInput")
    wt_in = nc.dram_tensor("wt_in", [P, WTC], f32, kind="ExternalInput")
    w0t_in = nc.dram_tensor("w0t_in", [D, D], f32, kind="ExternalInput")
    w1t_in = nc.dram_tensor("w1t_in", [D, D], f32, kind="ExternalInput")
    wot_in = nc.dram_tensor("wot_in", [D, 16], f32, kind="ExternalInput")
    pc_in = nc.dram_tensor("pc_in", [1, T * P], f32, kind="ExternalInput")
    out_dram = nc.dram_tensor("out", [P, T * 16], f32, kind="ExternalOutput")
    xt_shard = nc.dram_tensor("xt_shard", [P, T * D], f32)
    xt_table = nc.dram_tensor("xt_table", [N, D], f32, addr_space="Shared")
    groups = [list(range(C))]

    with tile.TileContext(nc) as tc:
        nc.gpsimd.load_library(library_config.mlp)
        import contextlib
        ctx = contextlib.ExitStack()
        with ctx:
            const = ctx.enter_context(tc.tile_pool(name="const", bufs=1))
            sqp = ctx.enter_context(tc.tile_pool(name="sq", bufs=3))
            xtp = ctx.enter_context(tc.tile_pool(name="xtp", bufs=3))
            gp = ctx.enter_context(tc.tile_pool(name="gp", bufs=3))
            scp = ctx.enter_context(tc.tile_pool(name="scp", bufs=2))
            psp = ctx.enter_context(tc.tile_pool(name="psp", bufs=2, space="PSUM"))
            psmv = ctx.enter_context(tc.tile_pool(name="psmv", bufs=2, space="PSUM"))

            ident = const.tile([P, P], f32)
            make_identity(nc, ident[:])
            idx_sb = const.tile([P, IDXC], i16)
            nc.sync.dma_start(out=idx_sb[:], in_=idx_in[:])
            wt_sb = const.tile([P, WTC], f32)
            nc.sync.dma_start(out=wt_sb[:], in_=wt_in[:])
            w0t_sb = const.tile([D, D], f32)
            nc.sync.dma_start(out=w0t_sb[:], in_=w0t_in[:])
            w1t_sb = const.tile([D, D], f32)
            nc.sync.dma_start(out=w1t_sb[:], in_=w1t_in[:])
            wot_sb = const.tile([D, 16], f32)
            nc.sync.dma_start(out=wot_sb[:], in_=wot_in[:])
            pc_sb = const.tile([1, T * P], f32)
            nc.sync.dma_start(out=pc_sb[:], in_=pc_in[:])

            x_sb = const.tile([P, T * D], f32)      # node state (manifold)
            mv_sb = const.tile([P, T * D], f32)     # W@x then xt (tangent msgs)
            agg_sb = const.tile([P, T * D], f32)    # aggregated tangent
            u_sb = const.tile([P, T * D], f32)      # relu'd tangent
            out_sb = const.tile([P, T * 16], f32)

            nc.sync.dma_start(out=x_sb[:], in_=h_in[:])

            def ts(t, w=D):
                return slice(t * w, (t + 1) * w)

            def artanh(dst, src):
                """dst = 0.5*ln((1+c)/(1-c)), c = min(src, ART_CLIP); src>=0."""
                cth = scp.tile([P, T], f32, tag="art_c")
                nc.vector.tensor_scalar_min(cth[:], src[:], ART_CLIP)
                pt = scp.tile([P, T], f32, tag="art_p")
                nc.scalar.activation(pt[:], cth[:], AF.Copy, bias=1.0)
                mt = scp.tile([P, T], f32, tag="art_m")
                nc.scalar.activation(mt[:], cth[:], AF.Copy, scale=-1.0, bias=1.0)
                rm = scp.tile([P, T], f32, tag="art_rm")
                nc.vector.reciprocal(rm[:], mt[:])
                nc.vector.tensor_tensor(pt[:], pt[:], rm[:], op=OP.mult)
                nc.scalar.activation(pt[:], pt[:], AF.Ln)
                nc.vector.tensor_scalar_mul(dst[:], pt[:], 0.5)

            def norm_from_sq(dst, src):
                nc.scalar.activation(dst[:], src[:], AF.Sqrt)
                nc.vector.tensor_scalar_max(dst[:], dst[:], EPS)

            def exp_proj_scale(dst, nrm):
                """dst = min(tanh(min(nrm,15)), MAXN) / nrm"""
                a = scp.tile([P, T], f32, tag="eps_a")
                nc.vector.tensor_scalar_min(a[:], nrm[:], MAX_TANH)
                nc.scalar.activation(a[:], a[:], AF.Tanh)
                nc.vector.tensor_scalar_min(a[:], a[:], MAXN)
                r = scp.tile([P, T], f32, tag="eps_r")
                nc.vector.reciprocal(r[:], nrm[:])
                nc.vector.tensor_tensor(dst[:], a[:], r[:], op=OP.mult)

            # ---- x0 = proj(expmap0(h)) --------------------------------------
            nh2 = scp.tile([P, T], f32, tag="nh2")
            for t in range(T):
                sq = sqp.tile([P, D], f32, tag="sq")
                nc.scalar.activation(sq[:], x_sb[:, ts(t)], AF.Square,
                                     accum_out=nh2[:, t:t + 1])
            nh = scp.tile([P, T], f32, tag="nh")
            norm_from_sq(nh, nh2)
            s0 = scp.tile([P, T], f32, tag="s0")
            exp_proj_scale(s0, nh)
            for t in range(T):
                nc.vector.tensor_scalar_mul(x_sb[:, ts(t)], x_sb[:, ts(t)],
                                            s0[:, t:t + 1])

            for layer in range(2):
                w_l = w0t_sb if layer == 0 else w1t_sb
                # ---- HypLinear + logmap0 (analytic combined scale) ----------
                xn2 = scp.tile([P, T], f32, tag="xn2")
                mxn2 = scp.tile([P, T], f32, tag="mxn2")
                for t in range(T):
                    sq = sqp.tile([P, D], f32, tag="sq")
                    nc.scalar.activation(sq[:], x_sb[:, ts(t)], AF.Square,
                                         accum_out=xn2[:, t:t + 1])
                    xT_ps = psp.tile([D, P], f32, tag="xT")
                    nc.tensor.transpose(out=xT_ps[:], in_=x_sb[:, ts(t)],
                                        identity=ident[:])
                    xT = xtp.tile([D, P], f32, tag="xT_sb")
                    nc.vector.tensor_copy(xT[:], xT_ps[:])
                    mv_ps = psmv.tile([P, D], f32, tag="mv")
                    nc.tensor.matmul(out=mv_ps[:], lhsT=xT[:], rhs=w_l[:],
                                     start=True, stop=True)
                    sq2 = sqp.tile([P, D], f32, tag="sq")
                    nc.scalar.activation(sq2[:], mv_ps[:], AF.Square,
                                         accum_out=mxn2[:, t:t + 1])
                    nc.vector.tensor_copy(mv_sb[:, ts(t)], mv_ps[:])
                xn = scp.tile([P, T], f32, tag="xn")
                norm_from_sq(xn, xn2)
                mxn = scp.tile([P, T], f32, tag="mxn")
                norm_from_sq(mxn, mxn2)
                at = scp.tile([P, T], f32, tag="at")
                artanh(at, xn)
                rx = scp.tile([P, T], f32, tag="rx")
                nc.vector.reciprocal(rx[:], xn[:])
                nc.vector.tensor_tensor(at[:], at[:], rx[:], op=OP.mult)
                nc.vector.tensor_tensor(at[:], at[:], mxn[:], op=OP.mult)
                nc.vector.tensor_scalar_min(at[:], at[:], MAX_TANH)
                nc.scalar.activation(at[:], at[:], AF.Tanh)
                nc.vector.tensor_scalar_min(at[:], at[:], MAXN)
                smsg = scp.tile([P, T], f32, tag="smsg")
                artanh(smsg, at)
                rmx = scp.tile([P, T], f32, tag="rmx")
                nc.vector.reciprocal(rmx[:], mxn[:])
                nc.vector.tensor_tensor(smsg[:], smsg[:], rmx[:], op=OP.mult)
                for t in range(T):
                    nc.vector.tensor_scalar_mul(mv_sb[:, ts(t)], mv_sb[:, ts(t)],
                                                smsg[:, t:t + 1])
                # publish shard and AllGather the full tangent table
                nc.sync.dma_start(out=xt_shard[:], in_=mv_sb[:])
                nobar = "nobar" in ablate
                if sim:
                    if not nobar:
                        tc.strict_bb_all_engine_barrier()
                    nc.sync.dma_start(
                        out=xt_table[0:NL, :].rearrange("(p x) d -> p x d", p=P),
                        in_=xt_shard[:].rearrange("p (x d) -> p x d", d=D))
                    if not nobar:
                        tc.strict_bb_all_engine_barrier()
                else:
                    if not nobar:
                        tc.strict_bb_all_engine_barrier()
                    nc.gpsimd.collective_compute(
                        "AllGather", mybir.AluOpType.bypass, replica_groups=groups,
                        ins=[xt_shard[:, :]], outs=[xt_table[:, :]])
                    if not nobar:
                        tc.strict_bb_all_engine_barrier()

                # ---- aggregation: gather + weighted reduce ------------------
                row0_sb = scp.tile([1, D], f32, tag="row0")
                if not use_wt:
                    nc.sync.dma_start(out=row0_sb[:], in_=xt_table[0:1, :])
                na2 = scp.tile([P, T], f32, tag="na2")
                ioff = woff = 0
                for t in range(T):
                    K = int(Ks[t])
                    g = gp.tile([P, K * D], f32, tag="G")
                    g3 = g[:].rearrange("p (k d) -> p k d", d=D)
                    if "gather" not in ablate:
                        nc.gpsimd.dma_gather(
                            g3, xt_table[:, :], idx_sb[:, ioff:ioff + 8 * K],
                            num_idxs=P * K, num_idxs_reg=P * K, elem_size=D,
                            single_packet=False)
                    if use_wt and "wtmul" not in ablate:
                        wt_ap = wt_sb[:, woff:woff + K]
                        wv = bass.AP(wt_ap.tensor, wt_ap.offset,
                                     list(wt_ap.ap) + [[0, D]])
                        nc.vector.tensor_tensor(g3, g3, wv, op=OP.mult)
                    if "reduce" not in ablate:
                        nc.vector.tensor_reduce(
                            agg_sb[:, ts(t)],
                            g[:].rearrange("p (k d) -> p d k", d=D),
                            axis=X, op=OP.add)
                    if not use_wt:
                        corr_ps = psmv.tile([P, D], f32, tag="mv")
                        nc.tensor.matmul(
                            out=corr_ps[:], lhsT=pc_sb[0:1, t * P:(t + 1) * P],
                            rhs=row0_sb[0:1, :], start=True, stop=True)
                        nc.vector.tensor_tensor(agg_sb[:, ts(t)],
                                                agg_sb[:, ts(t)], corr_ps[:],
                                                op=OP.subtract)
                    sq = sqp.tile([P, D], f32, tag="sq")
                    nc.scalar.activation(sq[:], agg_sb[:, ts(t)], AF.Square,
                                         accum_out=na2[:, t:t + 1])
                    ioff += 8 * K
                    woff += K
                # s2 = artanh(min(tanh(min(na,15)),MAXN)) / na
                na = scp.tile([P, T], f32, tag="na")
                norm_from_sq(na, na2)
                a2 = scp.tile([P, T], f32, tag="a2")
                nc.vector.tensor_scalar_min(a2[:], na[:], MAX_TANH)
                nc.scalar.activation(a2[:], a2[:], AF.Tanh)
                nc.vector.tensor_scalar_min(a2[:], a2[:], MAXN)
                s2 = scp.tile([P, T], f32, tag="s2")
                artanh(s2, a2)
                rna = scp.tile([P, T], f32, tag="rna")
                nc.vector.reciprocal(rna[:], na[:])
                nc.vector.tensor_tensor(s2[:], s2[:], rna[:], op=OP.mult)
                # u = relu(agg * s2); nu2 accum
                nu2 = scp.tile([P, T], f32, tag="nu2")
                for t in range(T):
                    nc.scalar.activation(u_sb[:, ts(t)], agg_sb[:, ts(t)],
                                         AF.Relu, scale=s2[:, t:t + 1])
                    sq = sqp.tile([P, D], f32, tag="sq")
                    nc.scalar.activation(sq[:], u_sb[:, ts(t)], AF.Square,
                                         accum_out=nu2[:, t:t + 1])
                nu = scp.tile([P, T], f32, tag="nu")
                norm_from_sq(nu, nu2)
                s3 = scp.tile([P, T], f32, tag="s3")
                exp_proj_scale(s3, nu)
                for t in range(T):
                    nc.vector.tensor_scalar_mul(x_sb[:, ts(t)], u_sb[:, ts(t)],
                                                s3[:, t:t + 1])

            # ---- readout: out = x @ W_out.T (b_out == 0) --------------------
            for t in range(T):
                xT_ps = psp.tile([D, P], f32, tag="xT")
                nc.tensor.transpose(out=xT_ps[:], in_=x_sb[:, ts(t)],
                                    identity=ident[:])
                xT = xtp.tile([D, P], f32, tag="xT_sb")
                nc.vector.tensor_copy(xT[:], xT_ps[:])
                o_ps = psmv.tile([P, 16], f32, tag="mv")
                nc.tensor.matmul(out=o_ps[:], lhsT=xT[:], rhs=wot_sb[:],
                                 start=True, stop=True)
                nc.vector.tensor_copy(out_sb[:, ts(t, 16)], o_ps[:])
            nc.sync.dma_start(out=out_dram[:], in_=out_sb[:])
    nc.compile()
    return nc


def kernel(h, distances, rows, cols, node_mask, edge_mask,
           W0, b0, W1, b1, W_out, b_out, _trace=False):
    from concourse.bass_utils import run_bass_kernel_spmd

    h = np.asarray(h, dtype=np.float32)
    rows = np.asarray(rows).astype(np.int64)
    cols = np.asarray(cols).astype(np.int64)
    node_mask = np.asarray(node_mask, dtype=np.float32)
    edge_mask = np.asarray(edge_mask, dtype=np.float32)
    assert not np.any(np.asarray(b0)) and not np.any(np.asarray(b1)) and \
        not np.any(np.asarray(b_out)), "nonzero biases unsupported"

    perm, Ks, idx_dev, wt_dev, IDXC, WTC, pc_dev, allones = _build_tables(
        rows, cols, edge_mask, node_mask)

    hp = h[perm].reshape(C, T, P, D).transpose(0, 2, 1, 3).reshape(C, P, T * D)
    w0t = np.ascontiguousarray(np.asarray(W0, np.float32).T)
    w1t = np.ascontiguousarray(np.asarray(W1, np.float32).T)
    wot = np.ascontiguousarray(np.asarray(W_out, np.float32).T)

    nc = _build_program(Ks, IDXC, WTC, use_wt=not allones)
    in_maps = [{
        "h_in": np.ascontiguousarray(hp[c]),
        "idx_in": idx_dev[c],
        "wt_in": wt_dev[c],
        "w0t_in": w0t, "w1t_in": w1t, "wot_in": wot,
        "pc_in": pc_dev[c],
    } for c in range(C)]
    res = run_bass_kernel_spmd(nc, in_maps, list(range(C)), trace=_trace)
    od = np.stack([res.results[c]["out"] for c in range(C)])
    od = od.reshape(C, P, T, 16).transpose(0, 2, 1, 3).reshape(N, 16)
    out = np.empty((N, 16), np.float32)
    out[perm] = od
    if _trace:
        return out, res
    return out



# revision 16
# speedup vs baseline: 1.3103x; 1.3103x over previous
"""HGCN decoder on 8 trn2 NeuronCores.

Strategy: nodes are sorted by in-degree, grouped into 128-node tiles, and the
tiles are dealt round-robin across the 8 cores (graph-parallel by destination
node).  Each core:
  - runs the node-wise hyperbolic math on its 4096 nodes,
  - publishes its tangent-space table shard, AllGathers the full [32768, 64]
    table to DRAM,
  - aggregates messages with `dma_gather` (padded per-tile CSR) followed by a
    strided reduce on the vector engine,
  - finishes with the euclidean readout matmul.

Key algebraic simplifications vs the literal reference (all exact up to fp
rounding, validated against the jax reference):
  - proj-then-logmap0 collapses: artanh(min(tanh(r), 1-eps)) == min(r, R*)
    with R* = artanh(1-eps).  This removes every Ln/artanh evaluation and all
    but one final Tanh, so the scalar chains are pure min/mult/recip/sqrt.
  - positive per-node scales commute through relu and matmuls, so the vector
    state is kept UNSCALED (raw) and only combined per-node scalars are
    carried between stages.  The only materialized scalings are the published
    message table (xt = mv_raw * sigma) and the final readout.
Activation-table note: the whole kernel needs only {Square, Sqrt, Relu, Copy}
(one table) plus a single trailing Tanh (second table) => 2 table loads total.

All graph preprocessing happens host-side in numpy; the device only sees
dense tables.
"""

import numpy as np

N = 32768
E = 1015808
D = 64
C = 8          # cores
NL = N // C    # 4096 nodes per core
P = 128        # partitions / tile
T = NL // P    # 32 tiles per core
Q = 8          # tiles per chain-quarter
NQ = T // Q
MAXN = 1.0 - 4e-3   # PROJ_EPS boundary for c=1
EPS = 1e-15
MAX_TANH = 15.0
R_STAR = float(np.arctanh(np.float64(1.0 - 4e-3)))  # artanh(MAXN) = 3.10635...


def _build_tables(rows, cols, edge_mask, node_mask):
    """Permute nodes by degree, deal tiles round-robin to cores, and build the
    per-core padded gather tables (int16 indices wrapped the way
    InstDMAGatherAnt wants them) plus matching weight tables."""
    deg = np.bincount(rows, minlength=N)
    order = np.argsort(-deg, kind="stable")
    # global tile j -> core j%C, slot j//C ; permuted position of its p-th node
    perm = np.empty(N, dtype=np.int64)
    j = np.arange(N) // P                     # global tile of sorted rank r
    c = j % C
    t = j // C
    p = np.arange(N) % P
    perm[c * NL + t * P + p] = order          # perm[g] = original node id
    pos = np.empty(N, dtype=np.int64)
    pos[perm] = np.arange(N)

    # gather-table row id for permuted position g=(c,t,p):
    #   AllGather concatenates per-core [P, T*D] blocks, so
    #   row_id = c*NL + p*T + t
    gg = np.arange(N)
    gc, gr = gg // NL, gg % NL
    gt, gp_ = gr // P, gr % P
    rowid = gc * NL + gp_ * T + gt            # [g] -> table row
    dstpos = pos[rows]
    eorder = np.argsort(dstpos, kind="stable")
    src_sorted = rowid[pos[cols[eorder]]]     # gather table rows, 0..N-1
    w_sorted = edge_mask[eorder, 0].astype(np.float64)
    cnts = np.bincount(dstpos, minlength=N)
    offs = np.zeros(N + 1, dtype=np.int64)
    np.cumsum(cnts, out=offs[1:])

    # per-slot K: max count over the 8 cores' tiles in that slot
    cnts_g = cnts.reshape(C, T, P)
    Ks = np.maximum(cnts_g.max(axis=(0, 2)), 1).astype(np.int64)   # [T]

    IDXC = int(8 * Ks.sum())
    WTC = int(Ks.sum())
    idx_dev = np.zeros((C, P, IDXC), np.int16)
    wt_dev = np.zeros((C, P, WTC), np.float32)
    nm = node_mask[:, 0].astype(np.float64)
    ioff = woff = 0
    ar = None
    for t in range(T):
        K = int(Ks[t])
        if ar is None or ar.shape[1] != K:
            ar = np.arange(K)[None, :]
        for cc in range(C):
            base = cc * NL + t * P
            cn = cnts[base:base + P]
            take = offs[base:base + P][:, None] + ar          # [P, K]
            valid = ar < cn[:, None]
            take_c = np.minimum(take, E - 1)
            nb = np.where(valid, src_sorted[take_c], 0)
            wl = np.where(valid, w_sorted[take_c], 0.0)
            wl = wl * nm[perm[base:base + P]][:, None]
            il = nb.T.reshape(-1)                             # i = g*128+p
            ch = il.reshape(8 * K, 16).T                      # [16, 8K]
            idx_dev[cc, :, ioff:ioff + 8 * K] = np.tile(ch, (8, 1)).astype(np.int16)
            wt_dev[cc, :, woff:woff + K] = wl.astype(np.float32)
        ioff += 8 * K
        woff += K
    # pad counts per (core, slot, partition) for the pad-subtract path
    pc_dev = np.zeros((C, 1, T * P), np.float32)
    for t in range(T):
        K = int(Ks[t])
        for cc in range(C):
            base = cc * NL + t * P
            pc_dev[cc, 0, t * P:(t + 1) * P] = K - cnts[base:base + P]
    allones = bool(np.all(edge_mask == 1.0) and np.all(node_mask == 1.0))
    return perm, Ks, idx_dev, wt_dev, IDXC, WTC, pc_dev, allones


def _build_program(Ks, IDXC, WTC, use_wt=True, sim=False):
    import concourse.bacc as bacc
    import concourse.bass as bass
    import concourse.mybir as mybir
    import concourse.tile as tile
    from concourse import library_config
    from concourse.masks import make_identity

    f32 = mybir.dt.float32
    i16 = mybir.dt.int16
    AF = mybir.ActivationFunctionType
    OP = mybir.AluOpType
    X = mybir.AxisListType.X

    nc = bacc.Bacc("TRN2", target_bir_lowering=False, debug=False,
                   num_devices=1 if sim else C)

    h_in = nc.dram_tensor("h_in", [P, T * D], f32, kind="ExternalInput")
    idx_in = nc.dram_tensor("idx_in", [P, IDXC], i16, kind="ExternalInput")
    wt_in = nc.dram_tensor("wt_in", [P, WTC], f32, kind="ExternalInput")
    w0t_in = nc.dram_tensor("w0t_in", [2 * D, D], f32, kind="ExternalInput")
    w1t_in = nc.dram_tensor("w1t_in", [2 * D, D], f32, kind="ExternalInput")
    wot_in = nc.dram_tensor("wot_in", [2 * D, 16], f32, kind="ExternalInput")
    pc_in = nc.dram_tensor("pc_in", [1, T * P], f32, kind="ExternalInput")
    out_dram = nc.dram_tensor("out", [P, T * 16], f32, kind="ExternalOutput")
    xt_shard = nc.dram_tensor("xt_shard", [P, T * D], f32)
    xt_table = nc.dram_tensor("xt_table", [N, D], f32, addr_space="Shared")
    groups = [list(range(C))]

    with tile.TileContext(nc) as tc:
        nc.gpsimd.load_library(library_config.mlp)
        import contextlib
        ctx = contextlib.ExitStack()
        with ctx:
            const = ctx.enter_context(tc.tile_pool(name="const", bufs=1))
            sqp = ctx.enter_context(tc.tile_pool(name="sq", bufs=2))
            xtp = ctx.enter_context(tc.tile_pool(name="xtp", bufs=3))
            gp = ctx.enter_context(tc.tile_pool(name="gp", bufs=4))
            scp = ctx.enter_context(tc.tile_pool(name="scp", bufs=3))
            psT = ctx.enter_context(tc.tile_pool(name="psT", bufs=2, space="PSUM"))
            psM = ctx.enter_context(tc.tile_pool(name="psM", bufs=2, space="PSUM"))
            psC = ctx.enter_context(tc.tile_pool(name="psC", bufs=2, space="PSUM"))
            psO = ctx.enter_context(tc.tile_pool(name="psO", bufs=2, space="PSUM"))

            ident = const.tile([P, P], f32)
            make_identity(nc, ident[:])
            idx_sb = const.tile([P, IDXC], i16)
            nc.sync.dma_start(out=idx_sb[:], in_=idx_in[:])
            if use_wt:
                wt_sb = const.tile([P, WTC], f32)
                nc.sync.dma_start(out=wt_sb[:], in_=wt_in[:])
            # weights duplicated across both partition halves so matmuls can
            # source lhsT from either half of a paired transpose
            w0t_sb = const.tile([2 * D, D], f32)
            nc.sync.dma_start(out=w0t_sb[:], in_=w0t_in[:])
            w1t_sb = const.tile([2 * D, D], f32)
            nc.sync.dma_start(out=w1t_sb[:], in_=w1t_in[:])
            wot_sb = const.tile([2 * D, 16], f32)
            nc.sync.dma_start(out=wot_sb[:], in_=wot_in[:])
            pc_sb = const.tile([1, T * P], f32)
            nc.sync.dma_start(out=pc_sb[:], in_=pc_in[:])

            u_sb = const.tile([P, T * D], f32)      # raw vector state / agg
            mv_sb = const.tile([P, T * D], f32)     # W@u then scaled messages
            out_sb = const.tile([P, T * 16], f32)
            # long-lived per-node scalars, [P, T] column t = tile t
            A_sb = const.tile([P, T], f32)    # artanh(||x||) of current state
            rn_sb = const.tile([P, T], f32)   # 1 / ||u_raw||
            t2_sb = const.tile([P, T], f32)   # final tanh argument (layer 2)
            na2_sb = const.tile([P, T], f32)
            nu2_sb = const.tile([P, T], f32)
            mx2_sb = const.tile([P, T], f32)
            sg_sb = const.tile([P, T], f32)   # publish scale sigma per tile
            sf_sb = const.tile([P, T], f32)   # final readout scale

            nc.sync.dma_start(out=u_sb[:], in_=h_in[:])

            def ts(t, w=D):
                return slice(t * w, (t + 1) * w)

            def act(out, in_, f, **kw):
                nc.scalar.activation(out, in_, f, **kw)

            # --- intro: norms of h; A = min(||h||, R*), rn = 1/||h|| --------
            n02 = scp.tile([P, T], f32, tag="n02")
            for q in range(NQ):
                sqb = sqp.tile([P, Q * D], f32, tag="sqb")
                act(sqb[:], u_sb[:, ts(q, Q * D)], AF.Square)
                nc.vector.tensor_reduce(
                    n02[:, ts(q, Q)],
                    sqb[:].rearrange("p (t d) -> p t d", d=D),
                    axis=X, op=OP.add)
            n0 = scp.tile([P, T], f32, tag="n0")
            act(n0[:], n02[:], AF.Sqrt)
            nc.vector.tensor_scalar_max(n0[:], n0[:], EPS)
            nc.vector.tensor_scalar_min(A_sb[:], n0[:], R_STAR)
            nc.vector.reciprocal(rn_sb[:], n0[:])

            def linear_block(w_sb, tiles, publish=True):
                """mv_raw = u @ W^T for `tiles`; mx2 = ||mv||^2; then
                sigma = min(A*mraw*rn, R*)/mraw; mv *= sigma; publish."""
                t0, t1 = tiles.start, tiles.stop
                nt = t1 - t0
                for j in range(nt // 2):
                    ta = t0 + 2 * j
                    ps = psT.tile([P, P], f32, tag="xT2")
                    nc.tensor.transpose(out=ps[:], in_=u_sb[:, ta * D:(ta + 2) * D],
                                        identity=ident[:])
                    xT2 = xtp.tile([P, P], f32, tag="xT2sb")
                    nc.vector.tensor_copy(xT2[:], ps[:])
                    for h2 in range(2):
                        t = ta + h2
                        mm = psM.tile([P, D], f32, tag="mv")
                        nc.tensor.matmul(out=mm[:], lhsT=xT2[h2 * D:(h2 + 1) * D, :],
                                         rhs=w_sb[h2 * D:(h2 + 1) * D, :],
                                         start=True, stop=True)
                        act(mv_sb[:, ts(t)], mm[:], AF.Copy)
                # batched ||mv||^2 then the sigma chain for these tiles
                for j in range((nt + Q - 1) // Q):
                    c0 = t0 + j * Q
                    c1 = min(c0 + Q, t1)
                    sqb = sqp.tile([P, Q * D], f32, tag="sqb")
                    act(sqb[:, :(c1 - c0) * D], mv_sb[:, c0 * D:c1 * D], AF.Square)
                    nc.vector.tensor_reduce(
                        mx2_sb[:, c0:c1],
                        sqb[:, :(c1 - c0) * D].rearrange("p (t d) -> p t d", d=D),
                        axis=X, op=OP.add)
                cols = slice(t0, t1)
                mraw = scp.tile([P, T], f32, tag="mraw")
                act(mraw[:, cols], mx2_sb[:, cols], AF.Sqrt)
                nc.vector.tensor_scalar_max(mraw[:, cols], mraw[:, cols], EPS)
                r = scp.tile([P, T], f32, tag="r")
                nc.vector.tensor_tensor(r[:, cols], A_sb[:, cols], mraw[:, cols],
                                        op=OP.mult)
                nc.vector.tensor_tensor(r[:, cols], r[:, cols], rn_sb[:, cols],
                                        op=OP.mult)
                nc.vector.tensor_scalar_min(r[:, cols], r[:, cols], R_STAR)
                rm = scp.tile([P, T], f32, tag="rm")
                nc.vector.reciprocal(rm[:, cols], mraw[:, cols])
                nc.vector.tensor_tensor(sg_sb[:, cols], r[:, cols], rm[:, cols],
                                        op=OP.mult)
                # mv *= sigma, broadcast over d in one strided multiply
                mv3 = mv_sb[:, t0 * D:t1 * D].rearrange("p (t d) -> p t d", d=D)
                sg_ap = sg_sb[:, cols]
                sgv = bass.AP(sg_ap.tensor, sg_ap.offset,
                              list(sg_ap.ap) + [[0, D]])
                nc.vector.tensor_tensor(mv3, mv3, sgv, op=OP.mult)
                if publish:
                    nc.sync.dma_start(out=xt_shard[:, t0 * D:t1 * D],
                                      in_=mv_sb[:, t0 * D:t1 * D])

            def readout_block(tiles):
                t0, t1 = tiles.start, tiles.stop
                for j in range((t1 - t0) // 2):
                    ta = t0 + 2 * j
                    ps = psT.tile([P, P], f32, tag="xT2")
                    nc.tensor.transpose(out=ps[:], in_=u_sb[:, ta * D:(ta + 2) * D],
                                        identity=ident[:])
                    xT2 = xtp.tile([P, P], f32, tag="xT2sb")
                    nc.vector.tensor_copy(xT2[:], ps[:])
                    for h2 in range(2):
                        t = ta + h2
                        po = psO.tile([P, 16], f32, tag="o")
                        nc.tensor.matmul(out=po[:], lhsT=xT2[h2 * D:(h2 + 1) * D, :],
                                         rhs=wot_sb[h2 * D:(h2 + 1) * D, :],
                                         start=True, stop=True)
                        act(out_sb[:, ts(t, 16)], po[:], AF.Copy)

            def allgather():
                tc.strict_bb_all_engine_barrier()
                if sim:
                    nc.sync.dma_start(
                        out=xt_table[0:NL, :].rearrange("(p x) d -> p x d", p=P),
                        in_=xt_shard[:].rearrange("p (x d) -> p x d", d=D))
                else:
                    nc.gpsimd.collective_compute(
                        "AllGather", mybir.AluOpType.bypass, replica_groups=groups,
                        ins=[xt_shard[:, :]], outs=[xt_table[:, :]])
                tc.strict_bb_all_engine_barrier()

            def post_agg_quarter(q, last_layer):
                """norms of agg & relu(agg); s' chain; t = nu*s'; A'/rn' (or
                stash t2 for the final tanh)."""
                cols = slice(q * Q, (q + 1) * Q)
                csl = slice(q * Q * D, (q + 1) * Q * D)
                sqb = sqp.tile([P, Q * D], f32, tag="sqb")
                act(sqb[:], u_sb[:, csl], AF.Square)
                nc.vector.tensor_reduce(
                    na2_sb[:, cols], sqb[:].rearrange("p (t d) -> p t d", d=D),
                    axis=X, op=OP.add)
                act(u_sb[:, csl], u_sb[:, csl], AF.Relu)
                sqb2 = sqp.tile([P, Q * D], f32, tag="sqb")
                act(sqb2[:], u_sb[:, csl], AF.Square)
                nc.vector.tensor_reduce(
                    nu2_sb[:, cols], sqb2[:].rearrange("p (t d) -> p t d", d=D),
                    axis=X, op=OP.add)
                na = scp.tile([P, Q], f32, tag="na")
                act(na[:], na2_sb[:, cols], AF.Sqrt)
                nc.vector.tensor_scalar_max(na[:], na[:], EPS)
                sp = scp.tile([P, Q], f32, tag="sp")
                nc.vector.tensor_scalar_min(sp[:], na[:], R_STAR)
                rna = scp.tile([P, Q], f32, tag="rna")
                nc.vector.reciprocal(rna[:], na[:])
                nc.vector.tensor_tensor(sp[:], sp[:], rna[:], op=OP.mult)
                nu = scp.tile([P, Q], f32, tag="nu")
                act(nu[:], nu2_sb[:, cols], AF.Sqrt)
                nc.vector.tensor_scalar_max(nu[:], nu[:], EPS)
                nc.vector.reciprocal(rn_sb[:, cols], nu[:])
                tq = scp.tile([P, Q], f32, tag="tq")
                nc.vector.tensor_tensor(tq[:], nu[:], sp[:], op=OP.mult)
                if last_layer:
                    nc.vector.tensor_scalar_min(t2_sb[:, cols], tq[:], MAX_TANH)
                else:
                    nc.vector.tensor_scalar_min(A_sb[:, cols], tq[:], R_STAR)

            def gather_layer(layer):
                """aggregate messages; per finished quarter run the post-agg
                chain and the next stage's linear work."""
                row0_sb = scp.tile([1, D], f32, tag="row0")
                if not use_wt:
                    nc.sync.dma_start(out=row0_sb[:], in_=xt_table[0:1, :])
                ioff = woff = 0
                for t in range(T):
                    K = int(Ks[t])
                    g = gp.tile([P, K * D], f32, tag="G")
                    g3 = g[:].rearrange("p (k d) -> p k d", d=D)
                    nc.gpsimd.dma_gather(
                        g3, xt_table[:, :], idx_sb[:, ioff:ioff + 8 * K],
                        num_idxs=P * K, num_idxs_reg=P * K, elem_size=D,
                        single_packet=False)
                    if use_wt:
                        wt_ap = wt_sb[:, woff:woff + K]
                        wv = bass.AP(wt_ap.tensor, wt_ap.offset,
                                     list(wt_ap.ap) + [[0, D]])
                        nc.vector.tensor_tensor(g3, g3, wv, op=OP.mult)
                    nc.vector.tensor_reduce(
                        u_sb[:, ts(t)],
                        g[:].rearrange("p (k d) -> p d k", d=D),
                        axis=X, op=OP.add)
                    if not use_wt:
                        corr_ps = psC.tile([P, D], f32, tag="corr")
                        nc.tensor.matmul(
                            out=corr_ps[:], lhsT=pc_sb[0:1, t * P:(t + 1) * P],
                            rhs=row0_sb[0:1, :], start=True, stop=True)
                        nc.vector.tensor_tensor(u_sb[:, ts(t)], u_sb[:, ts(t)],
                                                corr_ps[:], op=OP.subtract)
                    ioff += 8 * K
                    woff += K
                    if t % Q == Q - 1:
                        q = t // Q
                        post_agg_quarter(q, last_layer=(layer == 1))
                        if layer == 0:
                            linear_block(w1t_sb, slice(q * Q, (q + 1) * Q))
                        else:
                            readout_block(slice(q * Q, (q + 1) * Q))

            # ---- layer 1 linear + publish -----------------------------------
            linear_block(w0t_sb, slice(0, T))
            allgather()
            gather_layer(0)
            allgather()
            gather_layer(1)

            # ---- tail: single Tanh, final scale broadcast, store ------------
            th = scp.tile([P, T], f32, tag="th")
            act(th[:], t2_sb[:], AF.Tanh)
            nc.vector.tensor_scalar_min(th[:], th[:], MAXN)
            nc.vector.tensor_tensor(sf_sb[:], th[:], rn_sb[:], op=OP.mult)
            o3 = out_sb[:].rearrange("p (t j) -> p t j", j=16)
            sf_ap = sf_sb[:]
            sfv = bass.AP(sf_ap.tensor, sf_ap.offset, list(sf_ap.ap) + [[0, 16]])
            nc.vector.tensor_tensor(o3, o3, sfv, op=OP.mult)
            nc.sync.dma_start(out=out_dram[:], in_=out_sb[:])
    nc.compile()
    return nc


def kernel(h, distances, rows, cols, node_mask, edge_mask,
           W0, b0, W1, b1, W_out, b_out, _trace=False):
    from concourse.bass_utils import run_bass_kernel_spmd

    h = np.asarray(h, dtype=np.float32)
    rows = np.asarray(rows).astype(np.int64)
    cols = np.asarray(cols).astype(np.int64)
    node_mask = np.asarray(node_mask, dtype=np.float32)
    edge_mask = np.asarray(edge_mask, dtype=np.float32)
    assert not np.any(np.asarray(b0)) and not np.any(np.asarray(b1)) and \
        not np.any(np.asarray(b_out)), "nonzero biases unsupported"

    perm, Ks, idx_dev, wt_dev, IDXC, WTC, pc_dev, allones = _build_tables(
        rows, cols, edge_mask, node_mask)

    hp = h[perm].reshape(C, T, P, D).transpose(0, 2, 1, 3).reshape(C, P, T * D)
    w0t = np.ascontiguousarray(np.vstack([np.asarray(W0, np.float32).T] * 2))
    w1t = np.ascontiguousarray(np.vstack([np.asarray(W1, np.float32).T] * 2))
    wot = np.ascontiguousarray(np.vstack([np.asarray(W_out, np.float32).T] * 2))

    nc = _build_program(Ks, IDXC, WTC, use_wt=not allones)
    in_maps = [{
        "h_in": np.ascontiguousarray(hp[c]),
        "idx_in": idx_dev[c],
        "wt_in": wt_dev[c],
        "w0t_in": w0t, "w1t_in": w1t, "wot_in": wot,
        "pc_in": pc_dev[c],
    } for c in range(C)]
    res = run_bass_kernel_spmd(nc, in_maps, list(range(C)), trace=_trace)
    od = np.stack([res.results[c]["out"] for c in range(C)])
    od = od.reshape(C, P, T, 16).transpose(0, 2, 1, 3).reshape(N, 16)
    out = np.empty((N, 16), np.float32)
    out[perm] = od
    if _trace:
        return out, res
    return out


# revision 29
# speedup vs baseline: 1.3569x; 1.0356x over previous
"""HGCN decoder on 8 trn2 NeuronCores.

Strategy: nodes are sorted by in-degree, grouped into 128-node tiles, and the
tiles are dealt round-robin across the 8 cores (graph-parallel by destination
node).  Each core:
  - runs the node-wise hyperbolic math on its 4096 nodes,
  - publishes its tangent-space table shard, AllGathers the full [32768, 64]
    table to DRAM,
  - aggregates messages with `dma_gather` (padded per-tile CSR) followed by a
    strided reduce on the vector engine,
  - finishes with the euclidean readout matmul.

Key algebraic simplifications vs the literal reference (all exact up to fp
rounding, validated against the jax reference):
  - proj-then-logmap0 collapses: artanh(min(tanh(r), 1-eps)) == min(r, R*)
    with R* = artanh(1-eps).  This removes every Ln/artanh evaluation and all
    but one final Tanh, so the scalar chains are pure min/mult/recip/sqrt.
  - positive per-node scales commute through relu and matmuls, so the vector
    state is kept UNSCALED (raw) and only combined per-node scalars are
    carried between stages.  The only materialized scalings are the published
    message table (xt = mv_raw * sigma) and the final readout.
Activation-table note: the whole kernel needs only {Square, Sqrt, Relu, Copy}
(one table) plus a single trailing Tanh (second table) => 2 table loads total.

All graph preprocessing happens host-side in numpy; the device only sees
dense tables.
"""

import numpy as np

N = 32768
E = 1015808
D = 64
C = 8          # cores
NL = N // C    # 4096 nodes per core
P = 128        # partitions / tile
T = NL // P    # 32 tiles per core
Q = 8          # tiles per chain-quarter
NQ = T // Q
MAXN = 1.0 - 4e-3   # PROJ_EPS boundary for c=1
EPS = 1e-15
MAX_TANH = 15.0
R_STAR = float(np.arctanh(np.float64(1.0 - 4e-3)))  # artanh(MAXN) = 3.10635...


def _build_tables(rows, cols, edge_mask, node_mask):
    """Permute nodes by degree, deal tiles round-robin to cores, and build the
    per-core padded gather tables (int16 indices wrapped the way
    InstDMAGatherAnt wants them) plus matching weight tables."""
    deg = np.bincount(rows, minlength=N)
    order = np.argsort(-deg, kind="stable")
    # global tile j -> core j%C, slot j//C ; permuted position of its p-th node
    perm = np.empty(N, dtype=np.int64)
    j = np.arange(N) // P                     # global tile of sorted rank r
    c = j % C
    t = j // C
    p = np.arange(N) % P
    perm[c * NL + t * P + p] = order          # perm[g] = original node id
    pos = np.empty(N, dtype=np.int64)
    pos[perm] = np.arange(N)

    # gather-table row id for permuted position g=(c,t,p):
    #   AllGather concatenates per-core [P, T*D] blocks, so
    #   row_id = c*NL + p*T + t
    gg = np.arange(N)
    gc, gr = gg // NL, gg % NL
    gt, gp_ = gr // P, gr % P
    rowid = gc * NL + gp_ * T + gt            # [g] -> table row
    dstpos = pos[rows]
    eorder = np.argsort(dstpos, kind="stable")
    src_sorted = rowid[pos[cols[eorder]]]     # gather table rows, 0..N-1
    w_sorted = edge_mask[eorder, 0].astype(np.float64)
    cnts = np.bincount(dstpos, minlength=N)
    offs = np.zeros(N + 1, dtype=np.int64)
    np.cumsum(cnts, out=offs[1:])

    # per-slot K: max count over the 8 cores' tiles in that slot
    cnts_g = cnts.reshape(C, T, P)
    Ks = np.maximum(cnts_g.max(axis=(0, 2)), 1).astype(np.int64)   # [T]

    IDXC = int(8 * Ks.sum())
    WTC = int(Ks.sum())
    idx_dev = np.zeros((C, P, IDXC), np.int16)
    wt_dev = np.zeros((C, P, WTC), np.float32)
    nm = node_mask[:, 0].astype(np.float64)
    ioff = woff = 0
    ar = None
    for t in range(T):
        K = int(Ks[t])
        if ar is None or ar.shape[1] != K:
            ar = np.arange(K)[None, :]
        for cc in range(C):
            base = cc * NL + t * P
            cn = cnts[base:base + P]
            take = offs[base:base + P][:, None] + ar          # [P, K]
            valid = ar < cn[:, None]
            take_c = np.minimum(take, E - 1)
            nb = np.where(valid, src_sorted[take_c], 0)
            wl = np.where(valid, w_sorted[take_c], 0.0)
            wl = wl * nm[perm[base:base + P]][:, None]
            il = nb.T.reshape(-1)                             # i = g*128+p
            ch = il.reshape(8 * K, 16).T                      # [16, 8K]
            idx_dev[cc, :, ioff:ioff + 8 * K] = np.tile(ch, (8, 1)).astype(np.int16)
            wt_dev[cc, :, woff:woff + K] = wl.astype(np.float32)
        ioff += 8 * K
        woff += K
    # pad counts per (core, slot, partition) for the pad-subtract path
    pc_dev = np.zeros((C, 1, T * P), np.float32)
    for t in range(T):
        K = int(Ks[t])
        for cc in range(C):
            base = cc * NL + t * P
            pc_dev[cc, 0, t * P:(t + 1) * P] = K - cnts[base:base + P]
    allones = bool(np.all(edge_mask == 1.0) and np.all(node_mask == 1.0))
    return perm, Ks, idx_dev, wt_dev, IDXC, WTC, pc_dev, allones


def _build_program(Ks, IDXC, WTC, use_wt=True, sim=False):
    import concourse.bacc as bacc
    import concourse.bass as bass
    import concourse.mybir as mybir
    import concourse.tile as tile
    from concourse import library_config
    from concourse.masks import make_identity

    f32 = mybir.dt.float32
    i16 = mybir.dt.int16
    AF = mybir.ActivationFunctionType
    OP = mybir.AluOpType
    X = mybir.AxisListType.X

    nc = bacc.Bacc("TRN2", target_bir_lowering=False, debug=False,
                   num_devices=1 if sim else C)

    h_in = nc.dram_tensor("h_in", [P, T * D], f32, kind="ExternalInput")
    idx_in = nc.dram_tensor("idx_in", [P, IDXC], i16, kind="ExternalInput")
    wt_in = nc.dram_tensor("wt_in", [P, WTC], f32, kind="ExternalInput")
    w0t_in = nc.dram_tensor("w0t_in", [2 * D, D], f32, kind="ExternalInput")
    w1t_in = nc.dram_tensor("w1t_in", [2 * D, D], f32, kind="ExternalInput")
    wot_in = nc.dram_tensor("wot_in", [2 * D, 16], f32, kind="ExternalInput")
    pc_in = nc.dram_tensor("pc_in", [1, T * P], f32, kind="ExternalInput")
    out_dram = nc.dram_tensor("out", [P, T * 16], f32, kind="ExternalOutput")
    xt_shard = nc.dram_tensor("xt_shard", [P, T * D], f32)
    xt_table = nc.dram_tensor("xt_table", [N, D], f32, addr_space="Shared")
    groups = [list(range(C))]

    with tile.TileContext(nc) as tc:
        nc.gpsimd.load_library(library_config.mlp)
        import contextlib
        ctx = contextlib.ExitStack()
        with ctx:
            const = ctx.enter_context(tc.tile_pool(name="const", bufs=1))
            sqp = ctx.enter_context(tc.tile_pool(name="sq", bufs=2))
            xtp = ctx.enter_context(tc.tile_pool(name="xtp", bufs=3))
            gp = ctx.enter_context(tc.tile_pool(name="gp", bufs=4))
            scp = ctx.enter_context(tc.tile_pool(name="scp", bufs=3))
            psT = ctx.enter_context(tc.tile_pool(name="psT", bufs=2, space="PSUM"))
            psM = ctx.enter_context(tc.tile_pool(name="psM", bufs=2, space="PSUM"))
            psC = ctx.enter_context(tc.tile_pool(name="psC", bufs=2, space="PSUM"))
            psO = ctx.enter_context(tc.tile_pool(name="psO", bufs=2, space="PSUM"))

            ident = const.tile([P, P], f32)
            make_identity(nc, ident[:])
            # weights duplicated across both partition halves so matmuls can
            # source lhsT from either half of a paired transpose
            w0t_sb = const.tile([2 * D, D], f32)
            w1t_sb = const.tile([2 * D, D], f32)
            wot_sb = const.tile([2 * D, 16], f32)
            idx_sb = const.tile([P, IDXC], i16)
            pc_sb = const.tile([1, T * P], f32)
            if use_wt:
                wt_sb = const.tile([P, WTC], f32)

            u_sb = const.tile([P, T * D], f32)      # raw vector state / agg
            mv_sb = const.tile([P, T * D], f32)     # W@u then scaled messages
            out_sb = const.tile([P, T * 16], f32)
            # long-lived per-node scalars, [P, T] column t = tile t
            A_sb = const.tile([P, T], f32)    # artanh(||x||) of current state
            rn_sb = const.tile([P, T], f32)   # 1 / ||u_raw||
            t2_sb = const.tile([P, T], f32)   # final tanh argument (layer 2)
            na2_sb = const.tile([P, T], f32)
            nu2_sb = const.tile([P, T], f32)
            mx2_sb = const.tile([P, T], f32)
            sg_sb = const.tile([P, T], f32)   # publish scale sigma per tile
            sf_sb = const.tile([P, T], f32)   # final readout scale

            # load order: h + W0 first (intro/linear need them), the gather
            # metadata afterwards so it streams in under the intro compute
            nc.sync.dma_start(out=u_sb[:], in_=h_in[:])
            nc.sync.dma_start(out=w0t_sb[:], in_=w0t_in[:])
            nc.sync.dma_start(out=idx_sb[:], in_=idx_in[:])
            nc.sync.dma_start(out=w1t_sb[:], in_=w1t_in[:])
            nc.sync.dma_start(out=wot_sb[:], in_=wot_in[:])
            nc.sync.dma_start(out=pc_sb[:], in_=pc_in[:])
            if use_wt:
                nc.sync.dma_start(out=wt_sb[:], in_=wt_in[:])

            def ts(t, w=D):
                return slice(t * w, (t + 1) * w)

            def act(out, in_, f, **kw):
                nc.scalar.activation(out, in_, f, **kw)

            # --- intro: norms of h; A = min(||h||, R*), rn = 1/||h|| --------
            n02 = scp.tile([P, T], f32, tag="n02")
            for q in range(NQ):
                sqb = sqp.tile([P, Q * D], f32, tag="sqb")
                act(sqb[:], u_sb[:, ts(q, Q * D)], AF.Square)
                nc.vector.tensor_reduce(
                    n02[:, ts(q, Q)],
                    sqb[:].rearrange("p (t d) -> p t d", d=D),
                    axis=X, op=OP.add)
            n0 = scp.tile([P, T], f32, tag="n0")
            act(n0[:], n02[:], AF.Sqrt)
            nc.vector.tensor_scalar_max(n0[:], n0[:], EPS)
            nc.vector.tensor_scalar_min(A_sb[:], n0[:], R_STAR)
            nc.vector.reciprocal(rn_sb[:], n0[:])

            def linear_block(w_sb, tiles, publish=True):
                """mv_raw = u @ W^T for `tiles`; mx2 = ||mv||^2; then
                sigma = min(A*mraw*rn, R*)/mraw; mv *= sigma; publish."""
                t0, t1 = tiles.start, tiles.stop
                nt = t1 - t0
                for j in range(nt // 2):
                    ta = t0 + 2 * j
                    ps = psT.tile([P, P], f32, tag="xT2")
                    nc.tensor.transpose(out=ps[:], in_=u_sb[:, ta * D:(ta + 2) * D],
                                        identity=ident[:])
                    xT2 = xtp.tile([P, P], f32, tag="xT2sb")
                    nc.vector.tensor_copy(xT2[:], ps[:])
                    for h2 in range(2):
                        t = ta + h2
                        mm = psM.tile([P, D], f32, tag="mv")
                        nc.tensor.matmul(out=mm[:], lhsT=xT2[h2 * D:(h2 + 1) * D, :],
                                         rhs=w_sb[h2 * D:(h2 + 1) * D, :],
                                         start=True, stop=True)
                        act(mv_sb[:, ts(t)], mm[:], AF.Copy)
                # batched ||mv||^2 then the sigma chain for these tiles
                for j in range((nt + Q - 1) // Q):
                    c0 = t0 + j * Q
                    c1 = min(c0 + Q, t1)
                    sqb = sqp.tile([P, Q * D], f32, tag="sqb")
                    act(sqb[:, :(c1 - c0) * D], mv_sb[:, c0 * D:c1 * D], AF.Square)
                    nc.vector.tensor_reduce(
                        mx2_sb[:, c0:c1],
                        sqb[:, :(c1 - c0) * D].rearrange("p (t d) -> p t d", d=D),
                        axis=X, op=OP.add)
                cols = slice(t0, t1)
                mraw = scp.tile([P, T], f32, tag="mraw")
                act(mraw[:, cols], mx2_sb[:, cols], AF.Sqrt)
                nc.vector.tensor_scalar_max(mraw[:, cols], mraw[:, cols], EPS)
                r = scp.tile([P, T], f32, tag="r")
                nc.vector.tensor_tensor(r[:, cols], A_sb[:, cols], mraw[:, cols],
                                        op=OP.mult)
                nc.vector.tensor_tensor(r[:, cols], r[:, cols], rn_sb[:, cols],
                                        op=OP.mult)
                nc.vector.tensor_scalar_min(r[:, cols], r[:, cols], R_STAR)
                rm = scp.tile([P, T], f32, tag="rm")
                nc.vector.reciprocal(rm[:, cols], mraw[:, cols])
                nc.vector.tensor_tensor(sg_sb[:, cols], r[:, cols], rm[:, cols],
                                        op=OP.mult)
                # mv *= sigma, broadcast over d in one strided multiply
                mv3 = mv_sb[:, t0 * D:t1 * D].rearrange("p (t d) -> p t d", d=D)
                sg_ap = sg_sb[:, cols]
                sgv = bass.AP(sg_ap.tensor, sg_ap.offset,
                              list(sg_ap.ap) + [[0, D]])
                nc.vector.tensor_tensor(mv3, mv3, sgv, op=OP.mult)
                if publish:
                    nc.sync.dma_start(out=xt_shard[:, t0 * D:t1 * D],
                                      in_=mv_sb[:, t0 * D:t1 * D])

            def readout_block(tiles):
                t0, t1 = tiles.start, tiles.stop
                for j in range((t1 - t0) // 2):
                    ta = t0 + 2 * j
                    ps = psT.tile([P, P], f32, tag="xT2")
                    nc.tensor.transpose(out=ps[:], in_=u_sb[:, ta * D:(ta + 2) * D],
                                        identity=ident[:])
                    xT2 = xtp.tile([P, P], f32, tag="xT2sb")
                    nc.vector.tensor_copy(xT2[:], ps[:])
                    for h2 in range(2):
                        t = ta + h2
                        po = psO.tile([P, 16], f32, tag="o")
                        nc.tensor.matmul(out=po[:], lhsT=xT2[h2 * D:(h2 + 1) * D, :],
                                         rhs=wot_sb[h2 * D:(h2 + 1) * D, :],
                                         start=True, stop=True)
                        act(out_sb[:, ts(t, 16)], po[:], AF.Copy)

            def allgather():
                tc.strict_bb_all_engine_barrier()
                if sim:
                    nc.sync.dma_start(
                        out=xt_table[0:NL, :].rearrange("(p x) d -> p x d", p=P),
                        in_=xt_shard[:].rearrange("p (x d) -> p x d", d=D))
                else:
                    nc.gpsimd.collective_compute(
                        "AllGather", mybir.AluOpType.bypass, replica_groups=groups,
                        ins=[xt_shard[:, :]], outs=[xt_table[:, :]])

            def post_agg_group(g0, g1, last_layer):
                """norms of agg & relu(agg); s' chain; t = nu*s'; A'/rn' (or
                stash t2 for the final tanh)."""
                w = g1 - g0
                cols = slice(g0, g1)
                csl = slice(g0 * D, g1 * D)
                sqb = sqp.tile([P, Q * D], f32, tag="sqb")
                act(sqb[:, :w * D], u_sb[:, csl], AF.Square)
                nc.vector.tensor_reduce(
                    na2_sb[:, cols],
                    sqb[:, :w * D].rearrange("p (t d) -> p t d", d=D),
                    axis=X, op=OP.add)
                act(u_sb[:, csl], u_sb[:, csl], AF.Relu)
                sqb2 = sqp.tile([P, Q * D], f32, tag="sqb")
                act(sqb2[:, :w * D], u_sb[:, csl], AF.Square)
                nc.vector.tensor_reduce(
                    nu2_sb[:, cols],
                    sqb2[:, :w * D].rearrange("p (t d) -> p t d", d=D),
                    axis=X, op=OP.add)
                na = scp.tile([P, Q], f32, tag="na")
                act(na[:, :w], na2_sb[:, cols], AF.Sqrt)
                nc.vector.tensor_scalar_max(na[:, :w], na[:, :w], EPS)
                sp = scp.tile([P, Q], f32, tag="sp")
                nc.vector.tensor_scalar_min(sp[:, :w], na[:, :w], R_STAR)
                rna = scp.tile([P, Q], f32, tag="rna")
                nc.vector.reciprocal(rna[:, :w], na[:, :w])
                nc.vector.tensor_tensor(sp[:, :w], sp[:, :w], rna[:, :w],
                                        op=OP.mult)
                nu = scp.tile([P, Q], f32, tag="nu")
                act(nu[:, :w], nu2_sb[:, cols], AF.Sqrt)
                nc.vector.tensor_scalar_max(nu[:, :w], nu[:, :w], EPS)
                nc.vector.reciprocal(rn_sb[:, cols], nu[:, :w])
                tq = scp.tile([P, Q], f32, tag="tq")
                nc.vector.tensor_tensor(tq[:, :w], nu[:, :w], sp[:, :w],
                                        op=OP.mult)
                if last_layer:
                    nc.vector.tensor_scalar_min(t2_sb[:, cols], tq[:, :w],
                                                MAX_TANH)
                else:
                    nc.vector.tensor_scalar_min(A_sb[:, cols], tq[:, :w],
                                                R_STAR)

            # chain-group boundaries: big groups early (hidden under DMA),
            # small groups at the tail to shrink the exposed serial end
            GROUPS = [(0, 8), (8, 16), (16, 24), (24, 26), (26, 28),
                      (28, 30), (30, 32)]
            def gather_layer(layer):
                """aggregate messages; per finished chain-group run the
                post-agg chain and the next stage's linear work."""
                row0_sb = scp.tile([1, D], f32, tag="row0")
                if not use_wt:
                    nc.sync.dma_start(out=row0_sb[:], in_=xt_table[0:1, :])
                ioff = woff = 0
                gidx = 0
                for t in range(T):
                    K = int(Ks[t])
                    g = gp.tile([P, K * D], f32, tag="G")
                    g3 = g[:].rearrange("p (k d) -> p k d", d=D)
                    nc.gpsimd.dma_gather(
                        g3, xt_table[:, :], idx_sb[:, ioff:ioff + 8 * K],
                        num_idxs=P * K, num_idxs_reg=P * K, elem_size=D,
                        single_packet=False)
                    if use_wt:
                        wt_ap = wt_sb[:, woff:woff + K]
                        wv = bass.AP(wt_ap.tensor, wt_ap.offset,
                                     list(wt_ap.ap) + [[0, D]])
                        nc.vector.tensor_tensor(g3, g3, wv, op=OP.mult)
                    nc.vector.tensor_reduce(
                        u_sb[:, ts(t)],
                        g[:].rearrange("p (k d) -> p d k", d=D),
                        axis=X, op=OP.add)
                    if not use_wt:
                        corr_ps = psC.tile([P, D], f32, tag="corr")
                        nc.tensor.matmul(
                            out=corr_ps[:], lhsT=pc_sb[0:1, t * P:(t + 1) * P],
                            rhs=row0_sb[0:1, :], start=True, stop=True)
                        nc.vector.tensor_tensor(u_sb[:, ts(t)], u_sb[:, ts(t)],
                                                corr_ps[:], op=OP.subtract)
                    ioff += 8 * K
                    woff += K
                    if t + 1 == GROUPS[gidx][1]:
                        g0, g1 = GROUPS[gidx]
                        gidx += 1
                        post_agg_group(g0, g1, last_layer=(layer == 1))
                        if layer == 0:
                            linear_block(w1t_sb, slice(g0, g1))
                        else:
                            readout_block(slice(g0, g1))

            # ---- layer 1 linear + publish -----------------------------------
            for q in range(NQ):
                linear_block(w0t_sb, slice(q * Q, (q + 1) * Q))
            allgather()
            gather_layer(0)
            allgather()
            gather_layer(1)

            # ---- tail: single Tanh, final scale broadcast, store ------------
            th = scp.tile([P, T], f32, tag="th")
            act(th[:], t2_sb[:], AF.Tanh)
            nc.vector.tensor_scalar_min(th[:], th[:], MAXN)
            nc.vector.tensor_tensor(sf_sb[:], th[:], rn_sb[:], op=OP.mult)
            o3 = out_sb[:].rearrange("p (t j) -> p t j", j=16)
            sf_ap = sf_sb[:]
            sfv = bass.AP(sf_ap.tensor, sf_ap.offset, list(sf_ap.ap) + [[0, 16]])
            nc.vector.tensor_tensor(o3, o3, sfv, op=OP.mult)
            nc.sync.dma_start(out=out_dram[:], in_=out_sb[:])
    nc.compile()
    return nc


def kernel(h, distances, rows, cols, node_mask, edge_mask,
           W0, b0, W1, b1, W_out, b_out, _trace=False):
    from concourse.bass_utils import run_bass_kernel_spmd

    h = np.asarray(h, dtype=np.float32)
    rows = np.asarray(rows).astype(np.int64)
    cols = np.asarray(cols).astype(np.int64)
    node_mask = np.asarray(node_mask, dtype=np.float32)
    edge_mask = np.asarray(edge_mask, dtype=np.float32)
    assert not np.any(np.asarray(b0)) and not np.any(np.asarray(b1)) and \
        not np.any(np.asarray(b_out)), "nonzero biases unsupported"

    perm, Ks, idx_dev, wt_dev, IDXC, WTC, pc_dev, allones = _build_tables(
        rows, cols, edge_mask, node_mask)

    hp = h[perm].reshape(C, T, P, D).transpose(0, 2, 1, 3).reshape(C, P, T * D)
    w0t = np.ascontiguousarray(np.vstack([np.asarray(W0, np.float32).T] * 2))
    w1t = np.ascontiguousarray(np.vstack([np.asarray(W1, np.float32).T] * 2))
    wot = np.ascontiguousarray(np.vstack([np.asarray(W_out, np.float32).T] * 2))

    nc = _build_program(Ks, IDXC, WTC, use_wt=not allones)
    in_maps = [{
        "h_in": np.ascontiguousarray(hp[c]),
        "idx_in": idx_dev[c],
        "wt_in": wt_dev[c],
        "w0t_in": w0t, "w1t_in": w1t, "wot_in": wot,
        "pc_in": pc_dev[c],
    } for c in range(C)]
    res = run_bass_kernel_spmd(nc, in_maps, list(range(C)), trace=_trace)
    od = np.stack([res.results[c]["out"] for c in range(C)])
    od = od.reshape(C, P, T, 16).transpose(0, 2, 1, 3).reshape(N, 16)
    out = np.empty((N, 16), np.float32)
    out[perm] = od
    if _trace:
        return out, res
    return out


# revision 34
# speedup vs baseline: 1.3634x; 1.0047x over previous
"""HGCN decoder on 8 trn2 NeuronCores.

Strategy: nodes are sorted by in-degree, grouped into 128-node tiles, and the
tiles are dealt round-robin across the 8 cores (graph-parallel by destination
node).  Each core:
  - runs the node-wise hyperbolic math on its 4096 nodes,
  - publishes its tangent-space table shard, AllGathers the full [32768, 64]
    table to DRAM,
  - aggregates messages with `dma_gather` (padded per-tile CSR) followed by a
    strided reduce on the vector engine,
  - finishes with the euclidean readout matmul.

Key algebraic simplifications vs the literal reference (all exact up to fp
rounding, validated against the jax reference):
  - proj-then-logmap0 collapses: artanh(min(tanh(r), 1-eps)) == min(r, R*)
    with R* = artanh(1-eps).  This removes every Ln/artanh evaluation and all
    but one final Tanh, so the scalar chains are pure min/mult/recip/sqrt.
  - positive per-node scales commute through relu and matmuls, so the vector
    state is kept UNSCALED (raw) and only combined per-node scalars are
    carried between stages.  The only materialized scalings are the published
    message table (xt = mv_raw * sigma) and the final readout.
Activation-table note: the whole kernel needs only {Square, Sqrt, Relu, Copy}
(one table) plus a single trailing Tanh (second table) => 2 table loads total.

All graph preprocessing happens host-side in numpy; the device only sees
dense tables.
"""

import numpy as np

N = 32768
E = 1015808
D = 64
C = 8          # cores
NL = N // C    # 4096 nodes per core
P = 128        # partitions / tile
T = NL // P    # 32 tiles per core
Q = 8          # tiles per chain-quarter
NQ = T // Q
MAXN = 1.0 - 4e-3   # PROJ_EPS boundary for c=1
EPS = 1e-15
MAX_TANH = 15.0
R_STAR = float(np.arctanh(np.float64(1.0 - 4e-3)))  # artanh(MAXN) = 3.10635...


def _build_tables(rows, cols, edge_mask, node_mask):
    """Permute nodes by degree, deal tiles round-robin to cores, and build the
    per-core padded gather tables (int16 indices wrapped the way
    InstDMAGatherAnt wants them) plus matching weight tables."""
    deg = np.bincount(rows, minlength=N)
    order = np.argsort(-deg, kind="stable")
    # global tile j -> core j%C, slot j//C ; permuted position of its p-th node
    perm = np.empty(N, dtype=np.int64)
    j = np.arange(N) // P                     # global tile of sorted rank r
    c = j % C
    t = j // C
    p = np.arange(N) % P
    perm[c * NL + t * P + p] = order          # perm[g] = original node id
    pos = np.empty(N, dtype=np.int64)
    pos[perm] = np.arange(N)

    # gather-table row id for permuted position g=(c,t,p):
    #   AllGather concatenates per-core [P, T*D] blocks, so
    #   row_id = c*NL + p*T + t
    gg = np.arange(N)
    gc, gr = gg // NL, gg % NL
    gt, gp_ = gr // P, gr % P
    rowid = gc * NL + gp_ * T + gt            # [g] -> table row
    dstpos = pos[rows]
    eorder = np.argsort(dstpos, kind="stable")
    src_sorted = rowid[pos[cols[eorder]]]     # gather table rows, 0..N-1
    w_sorted = edge_mask[eorder, 0].astype(np.float64)
    cnts = np.bincount(dstpos, minlength=N)
    offs = np.zeros(N + 1, dtype=np.int64)
    np.cumsum(cnts, out=offs[1:])

    # per-slot K: max count over the 8 cores' tiles in that slot
    cnts_g = cnts.reshape(C, T, P)
    Ks = np.maximum(cnts_g.max(axis=(0, 2)), 1).astype(np.int64)   # [T]

    IDXC = int(8 * Ks.sum())
    WTC = int(Ks.sum())
    idx_dev = np.zeros((C, P, IDXC), np.int16)
    wt_dev = np.zeros((C, P, WTC), np.float32)
    nm = node_mask[:, 0].astype(np.float64)
    ioff = woff = 0
    ar = None
    for t in range(T):
        K = int(Ks[t])
        if ar is None or ar.shape[1] != K:
            ar = np.arange(K)[None, :]
        for cc in range(C):
            base = cc * NL + t * P
            cn = cnts[base:base + P]
            take = offs[base:base + P][:, None] + ar          # [P, K]
            valid = ar < cn[:, None]
            take_c = np.minimum(take, E - 1)
            nb = np.where(valid, src_sorted[take_c], 0)
            wl = np.where(valid, w_sorted[take_c], 0.0)
            wl = wl * nm[perm[base:base + P]][:, None]
            il = nb.T.reshape(-1)                             # i = g*128+p
            ch = il.reshape(8 * K, 16).T                      # [16, 8K]
            idx_dev[cc, :, ioff:ioff + 8 * K] = np.tile(ch, (8, 1)).astype(np.int16)
            wt_dev[cc, :, woff:woff + K] = wl.astype(np.float32)
        ioff += 8 * K
        woff += K
    # pad counts per (core, slot, partition) for the pad-subtract path
    pc_dev = np.zeros((C, 1, T * P), np.float32)
    for t in range(T):
        K = int(Ks[t])
        for cc in range(C):
            base = cc * NL + t * P
            pc_dev[cc, 0, t * P:(t + 1) * P] = K - cnts[base:base + P]
    allones = bool(np.all(edge_mask == 1.0) and np.all(node_mask == 1.0))
    return perm, Ks, idx_dev, wt_dev, IDXC, WTC, pc_dev, allones


def _build_program(Ks, IDXC, WTC, use_wt=True, sim=False):
    import concourse.bacc as bacc
    import concourse.bass as bass
    import concourse.mybir as mybir
    import concourse.tile as tile
    from concourse import library_config
    from concourse.masks import make_identity

    f32 = mybir.dt.float32
    i16 = mybir.dt.int16
    AF = mybir.ActivationFunctionType
    OP = mybir.AluOpType
    X = mybir.AxisListType.X

    nc = bacc.Bacc("TRN2", target_bir_lowering=False, debug=False,
                   num_devices=1 if sim else C)

    h_in = nc.dram_tensor("h_in", [P, T * D], f32, kind="ExternalInput")
    idx_in = nc.dram_tensor("idx_in", [P, IDXC], i16, kind="ExternalInput")
    wt_in = nc.dram_tensor("wt_in", [P, WTC], f32, kind="ExternalInput")
    w0t_in = nc.dram_tensor("w0t_in", [2 * D, D], f32, kind="ExternalInput")
    w1t_in = nc.dram_tensor("w1t_in", [2 * D, D], f32, kind="ExternalInput")
    wot_in = nc.dram_tensor("wot_in", [2 * D, 16], f32, kind="ExternalInput")
    pc_in = nc.dram_tensor("pc_in", [1, T * P], f32, kind="ExternalInput")
    out_dram = nc.dram_tensor("out", [P, T * 16], f32, kind="ExternalOutput")
    xt_shard = nc.dram_tensor("xt_shard", [P, T * D], f32)
    xt_table = nc.dram_tensor("xt_table", [N, D], f32, addr_space="Shared")
    groups = [list(range(C))]

    with tile.TileContext(nc) as tc:
        nc.gpsimd.load_library(library_config.mlp)
        import contextlib
        ctx = contextlib.ExitStack()
        with ctx:
            const = ctx.enter_context(tc.tile_pool(name="const", bufs=1))
            sqp = ctx.enter_context(tc.tile_pool(name="sq", bufs=2))
            xtp = ctx.enter_context(tc.tile_pool(name="xtp", bufs=3))
            gp = ctx.enter_context(tc.tile_pool(name="gp", bufs=4))
            scp = ctx.enter_context(tc.tile_pool(name="scp", bufs=3))
            psT = ctx.enter_context(tc.tile_pool(name="psT", bufs=2, space="PSUM"))
            psM = ctx.enter_context(tc.tile_pool(name="psM", bufs=2, space="PSUM"))
            psC = ctx.enter_context(tc.tile_pool(name="psC", bufs=2, space="PSUM"))
            psO = ctx.enter_context(tc.tile_pool(name="psO", bufs=2, space="PSUM"))

            ident = const.tile([P, P], f32)
            make_identity(nc, ident[:])
            # weights duplicated across both partition halves so matmuls can
            # source lhsT from either half of a paired transpose
            w0t_sb = const.tile([2 * D, D], f32)
            w1t_sb = const.tile([2 * D, D], f32)
            wot_sb = const.tile([2 * D, 16], f32)
            idx_sb = const.tile([P, IDXC], i16)
            pc_sb = const.tile([1, T * P], f32)
            if use_wt:
                wt_sb = const.tile([P, WTC], f32)

            u_sb = const.tile([P, T * D], f32)      # raw vector state / agg
            mv_sb = const.tile([P, T * D], f32)     # W@u then scaled messages
            out_sb = const.tile([P, T * 16], f32)
            # long-lived per-node scalars, [P, T] column t = tile t
            A_sb = const.tile([P, T], f32)    # artanh(||x||) of current state
            rn_sb = const.tile([P, T], f32)   # 1 / ||u_raw||
            t2_sb = const.tile([P, T], f32)   # final tanh argument (layer 2)
            na2_sb = const.tile([P, T], f32)
            nu2_sb = const.tile([P, T], f32)
            mx2_sb = const.tile([P, T], f32)
            sg_sb = const.tile([P, T], f32)   # publish scale sigma per tile
            sf_sb = const.tile([P, T], f32)   # final readout scale

            # load order: h quarter 0 + W0 first (intro/linear need them), the
            # gather metadata afterwards so it streams under the intro compute
            QW = Q * D
            nc.sync.dma_start(out=u_sb[:, 0:QW], in_=h_in[:, 0:QW])
            nc.sync.dma_start(out=w0t_sb[:], in_=w0t_in[:])
            for q in range(1, NQ):
                nc.sync.dma_start(out=u_sb[:, q * QW:(q + 1) * QW],
                                  in_=h_in[:, q * QW:(q + 1) * QW])
            nc.sync.dma_start(out=w1t_sb[:], in_=w1t_in[:])
            nc.sync.dma_start(out=wot_sb[:], in_=wot_in[:])
            nc.sync.dma_start(out=pc_sb[:], in_=pc_in[:])
            nc.sync.dma_start(out=idx_sb[:], in_=idx_in[:])
            if use_wt:
                nc.sync.dma_start(out=wt_sb[:], in_=wt_in[:])

            def ts(t, w=D):
                return slice(t * w, (t + 1) * w)

            def act(out, in_, f, **kw):
                nc.scalar.activation(out, in_, f, **kw)

            def intro_quarter(q):
                """A = min(||h||, R*), rn = 1/||h|| for quarter q."""
                n02 = scp.tile([P, Q], f32, tag="n02")
                sqb = sqp.tile([P, Q * D], f32, tag="sqb")
                act(sqb[:], u_sb[:, ts(q, Q * D)], AF.Square)
                nc.vector.tensor_reduce(
                    n02[:], sqb[:].rearrange("p (t d) -> p t d", d=D),
                    axis=X, op=OP.add)
                n0 = scp.tile([P, Q], f32, tag="n0")
                act(n0[:], n02[:], AF.Sqrt)
                nc.vector.tensor_scalar_max(n0[:], n0[:], EPS)
                nc.vector.tensor_scalar_min(A_sb[:, ts(q, Q)], n0[:], R_STAR)
                nc.vector.reciprocal(rn_sb[:, ts(q, Q)], n0[:])

            def linear_block(w_sb, tiles, publish=True):
                """mv_raw = u @ W^T for `tiles`; mx2 = ||mv||^2; then
                sigma = min(A*mraw*rn, R*)/mraw; mv *= sigma; publish."""
                t0, t1 = tiles.start, tiles.stop
                nt = t1 - t0
                for j in range(nt // 2):
                    ta = t0 + 2 * j
                    ps = psT.tile([P, P], f32, tag="xT2")
                    nc.tensor.transpose(out=ps[:], in_=u_sb[:, ta * D:(ta + 2) * D],
                                        identity=ident[:])
                    xT2 = xtp.tile([P, P], f32, tag="xT2sb")
                    nc.vector.tensor_copy(xT2[:], ps[:])
                    for h2 in range(2):
                        t = ta + h2
                        mm = psM.tile([P, D], f32, tag="mv")
                        nc.tensor.matmul(out=mm[:], lhsT=xT2[h2 * D:(h2 + 1) * D, :],
                                         rhs=w_sb[h2 * D:(h2 + 1) * D, :],
                                         start=True, stop=True)
                        act(mv_sb[:, ts(t)], mm[:], AF.Copy)
                # batched ||mv||^2 then the sigma chain for these tiles
                for j in range((nt + Q - 1) // Q):
                    c0 = t0 + j * Q
                    c1 = min(c0 + Q, t1)
                    sqb = sqp.tile([P, Q * D], f32, tag="sqb")
                    act(sqb[:, :(c1 - c0) * D], mv_sb[:, c0 * D:c1 * D], AF.Square)
                    nc.vector.tensor_reduce(
                        mx2_sb[:, c0:c1],
                        sqb[:, :(c1 - c0) * D].rearrange("p (t d) -> p t d", d=D),
                        axis=X, op=OP.add)
                cols = slice(t0, t1)
                mraw = scp.tile([P, T], f32, tag="mraw")
                act(mraw[:, cols], mx2_sb[:, cols], AF.Sqrt)
                nc.vector.tensor_scalar_max(mraw[:, cols], mraw[:, cols], EPS)
                r = scp.tile([P, T], f32, tag="r")
                nc.vector.tensor_tensor(r[:, cols], A_sb[:, cols], mraw[:, cols],
                                        op=OP.mult)
                nc.vector.tensor_tensor(r[:, cols], r[:, cols], rn_sb[:, cols],
                                        op=OP.mult)
                nc.vector.tensor_scalar_min(r[:, cols], r[:, cols], R_STAR)
                rm = scp.tile([P, T], f32, tag="rm")
                nc.vector.reciprocal(rm[:, cols], mraw[:, cols])
                nc.vector.tensor_tensor(sg_sb[:, cols], r[:, cols], rm[:, cols],
                                        op=OP.mult)
                # mv *= sigma, broadcast over d in one strided multiply
                mv3 = mv_sb[:, t0 * D:t1 * D].rearrange("p (t d) -> p t d", d=D)
                sg_ap = sg_sb[:, cols]
                sgv = bass.AP(sg_ap.tensor, sg_ap.offset,
                              list(sg_ap.ap) + [[0, D]])
                nc.vector.tensor_tensor(mv3, mv3, sgv, op=OP.mult)
                if publish:
                    nc.sync.dma_start(out=xt_shard[:, t0 * D:t1 * D],
                                      in_=mv_sb[:, t0 * D:t1 * D])

            def readout_block(tiles):
                t0, t1 = tiles.start, tiles.stop
                for j in range((t1 - t0) // 2):
                    ta = t0 + 2 * j
                    ps = psT.tile([P, P], f32, tag="xT2")
                    nc.tensor.transpose(out=ps[:], in_=u_sb[:, ta * D:(ta + 2) * D],
                                        identity=ident[:])
                    xT2 = xtp.tile([P, P], f32, tag="xT2sb")
                    nc.vector.tensor_copy(xT2[:], ps[:])
                    for h2 in range(2):
                        t = ta + h2
                        po = psO.tile([P, 16], f32, tag="o")
                        nc.tensor.matmul(out=po[:], lhsT=xT2[h2 * D:(h2 + 1) * D, :],
                                         rhs=wot_sb[h2 * D:(h2 + 1) * D, :],
                                         start=True, stop=True)
                        act(out_sb[:, ts(t, 16)], po[:], AF.Copy)

            def allgather():
                if sim:
                    nc.sync.dma_start(
                        out=xt_table[0:NL, :].rearrange("(p x) d -> p x d", p=P),
                        in_=xt_shard[:].rearrange("p (x d) -> p x d", d=D))
                else:
                    nc.gpsimd.collective_compute(
                        "AllGather", mybir.AluOpType.bypass, replica_groups=groups,
                        ins=[xt_shard[:, :]], outs=[xt_table[:, :]])

            def post_agg_group(g0, g1, last_layer):
                """norms of agg & relu(agg); s' chain; t = nu*s'; A'/rn' (or
                stash t2 for the final tanh)."""
                w = g1 - g0
                cols = slice(g0, g1)
                csl = slice(g0 * D, g1 * D)
                sqb = sqp.tile([P, Q * D], f32, tag="sqb")
                act(sqb[:, :w * D], u_sb[:, csl], AF.Square)
                nc.vector.tensor_reduce(
                    na2_sb[:, cols],
                    sqb[:, :w * D].rearrange("p (t d) -> p t d", d=D),
                    axis=X, op=OP.add)
                act(u_sb[:, csl], u_sb[:, csl], AF.Relu)
                sqb2 = sqp.tile([P, Q * D], f32, tag="sqb")
                act(sqb2[:, :w * D], u_sb[:, csl], AF.Square)
                nc.vector.tensor_reduce(
                    nu2_sb[:, cols],
                    sqb2[:, :w * D].rearrange("p (t d) -> p t d", d=D),
                    axis=X, op=OP.add)
                na = scp.tile([P, Q], f32, tag="na")
                act(na[:, :w], na2_sb[:, cols], AF.Sqrt)
                nc.vector.tensor_scalar_max(na[:, :w], na[:, :w], EPS)
                sp = scp.tile([P, Q], f32, tag="sp")
                nc.vector.tensor_scalar_min(sp[:, :w], na[:, :w], R_STAR)
                rna = scp.tile([P, Q], f32, tag="rna")
                nc.vector.reciprocal(rna[:, :w], na[:, :w])
                nc.vector.tensor_tensor(sp[:, :w], sp[:, :w], rna[:, :w],
                                        op=OP.mult)
                nu = scp.tile([P, Q], f32, tag="nu")
                act(nu[:, :w], nu2_sb[:, cols], AF.Sqrt)
                nc.vector.tensor_scalar_max(nu[:, :w], nu[:, :w], EPS)
                nc.vector.reciprocal(rn_sb[:, cols], nu[:, :w])
                tq = scp.tile([P, Q], f32, tag="tq")
                nc.vector.tensor_tensor(tq[:, :w], nu[:, :w], sp[:, :w],
                                        op=OP.mult)
                if last_layer:
                    nc.vector.tensor_scalar_min(t2_sb[:, cols], tq[:, :w],
                                                MAX_TANH)
                else:
                    nc.vector.tensor_scalar_min(A_sb[:, cols], tq[:, :w],
                                                R_STAR)

            # chain-group boundaries: big groups early (hidden under DMA),
            # small groups at the tail to shrink the exposed serial end
            GROUPS = [(0, 8), (8, 16), (16, 24), (24, 26), (26, 28),
                      (28, 30), (30, 32)]
            # linear/readout work keeps pair alignment; chains can be finer
            def gather_layer(layer):
                """aggregate messages; per finished chain-group run the
                post-agg chain and the next stage's linear work."""
                row0_sb = scp.tile([1, D], f32, tag="row0")
                if not use_wt:
                    nc.sync.dma_start(out=row0_sb[:], in_=xt_table[0:1, :])
                ioff = woff = 0
                gidx = 0
                for t in range(T):
                    K = int(Ks[t])
                    g = gp.tile([P, K * D], f32, tag="G")
                    g3 = g[:].rearrange("p (k d) -> p k d", d=D)
                    nc.gpsimd.dma_gather(
                        g3, xt_table[:, :], idx_sb[:, ioff:ioff + 8 * K],
                        num_idxs=P * K, num_idxs_reg=P * K, elem_size=D,
                        single_packet=False)
                    if use_wt:
                        wt_ap = wt_sb[:, woff:woff + K]
                        wv = bass.AP(wt_ap.tensor, wt_ap.offset,
                                     list(wt_ap.ap) + [[0, D]])
                        nc.vector.tensor_tensor(g3, g3, wv, op=OP.mult)
                    nc.vector.tensor_reduce(
                        u_sb[:, ts(t)],
                        g[:].rearrange("p (k d) -> p d k", d=D),
                        axis=X, op=OP.add)
                    if not use_wt:
                        corr_ps = psC.tile([P, D], f32, tag="corr")
                        nc.tensor.matmul(
                            out=corr_ps[:], lhsT=pc_sb[0:1, t * P:(t + 1) * P],
                            rhs=row0_sb[0:1, :], start=True, stop=True)
                        nc.vector.tensor_tensor(u_sb[:, ts(t)], u_sb[:, ts(t)],
                                                corr_ps[:], op=OP.subtract)
                    ioff += 8 * K
                    woff += K
                    if t + 1 == GROUPS[gidx][1]:
                        g0, g1 = GROUPS[gidx]
                        gidx += 1
                        post_agg_group(g0, g1, last_layer=(layer == 1))
                        if layer == 0:
                            linear_block(w1t_sb, slice(g0, g1))
                        else:
                            readout_block(slice(g0, g1))

            # ---- layer 1 linear + publish -----------------------------------
            for q in range(NQ):
                intro_quarter(q)
                linear_block(w0t_sb, slice(q * Q, (q + 1) * Q))
            allgather()
            gather_layer(0)
            allgather()
            gather_layer(1)

            # ---- tail: single Tanh, final scale broadcast, store ------------
            th = scp.tile([P, T], f32, tag="th")
            act(th[:], t2_sb[:], AF.Tanh)
            nc.vector.tensor_scalar_min(th[:], th[:], MAXN)
            nc.vector.tensor_tensor(sf_sb[:], th[:], rn_sb[:], op=OP.mult)
            o3 = out_sb[:].rearrange("p (t j) -> p t j", j=16)
            sf_ap = sf_sb[:]
            sfv = bass.AP(sf_ap.tensor, sf_ap.offset, list(sf_ap.ap) + [[0, 16]])
            nc.vector.tensor_tensor(o3, o3, sfv, op=OP.mult)
            nc.sync.dma_start(out=out_dram[:], in_=out_sb[:])
    nc.compile()
    return nc


def kernel(h, distances, rows, cols, node_mask, edge_mask,
           W0, b0, W1, b1, W_out, b_out, _trace=False):
    from concourse.bass_utils import run_bass_kernel_spmd

    h = np.asarray(h, dtype=np.float32)
    rows = np.asarray(rows).astype(np.int64)
    cols = np.asarray(cols).astype(np.int64)
    node_mask = np.asarray(node_mask, dtype=np.float32)
    edge_mask = np.asarray(edge_mask, dtype=np.float32)
    assert not np.any(np.asarray(b0)) and not np.any(np.asarray(b1)) and \
        not np.any(np.asarray(b_out)), "nonzero biases unsupported"

    perm, Ks, idx_dev, wt_dev, IDXC, WTC, pc_dev, allones = _build_tables(
        rows, cols, edge_mask, node_mask)

    hp = h[perm].reshape(C, T, P, D).transpose(0, 2, 1, 3).reshape(C, P, T * D)
    w0t = np.ascontiguousarray(np.vstack([np.asarray(W0, np.float32).T] * 2))
    w1t = np.ascontiguousarray(np.vstack([np.asarray(W1, np.float32).T] * 2))
    wot = np.ascontiguousarray(np.vstack([np.asarray(W_out, np.float32).T] * 2))

    nc = _build_program(Ks, IDXC, WTC, use_wt=not allones)
    in_maps = [{
        "h_in": np.ascontiguousarray(hp[c]),
        "idx_in": idx_dev[c],
        "wt_in": wt_dev[c],
        "w0t_in": w0t, "w1t_in": w1t, "wot_in": wot,
        "pc_in": pc_dev[c],
    } for c in range(C)]
    res = run_bass_kernel_spmd(nc, in_maps, list(range(C)), trace=_trace)
    od = np.stack([res.results[c]["out"] for c in range(C)])
    od = od.reshape(C, P, T, 16).transpose(0, 2, 1, 3).reshape(N, 16)
    out = np.empty((N, 16), np.float32)
    out[perm] = od
    if _trace:
        return out, res
    return out


# revision 43
# speedup vs baseline: 1.3864x; 1.0169x over previous
"""HGCN decoder on 8 trn2 NeuronCores.

Strategy: nodes are sorted by in-degree, grouped into 128-node tiles, and the
tiles are dealt round-robin across the 8 cores (graph-parallel by destination
node).  Each core:
  - runs the node-wise hyperbolic math on its 4096 nodes,
  - publishes its tangent-space table shard, AllGathers the full [32768, 64]
    table to DRAM,
  - aggregates messages with `dma_gather` (padded per-tile CSR) followed by a
    strided reduce on the vector engine,
  - finishes with the euclidean readout matmul.

Key algebraic simplifications vs the literal reference (all exact up to fp
rounding, validated against the jax reference):
  - proj-then-logmap0 collapses: artanh(min(tanh(r), 1-eps)) == min(r, R*)
    with R* = artanh(1-eps).  This removes every Ln/artanh evaluation and all
    but one final Tanh, so the scalar chains are pure min/mult/recip/sqrt.
  - positive per-node scales commute through relu and matmuls, so the vector
    state is kept UNSCALED (raw) and only combined per-node scalars are
    carried between stages.  The only materialized scalings are the published
    message table (xt = mv_raw * sigma) and the final readout.
Activation-table note: the whole kernel needs only {Square, Sqrt, Relu, Copy}
(one table) plus a single trailing Tanh (second table) => 2 table loads total.

All graph preprocessing happens host-side in numpy; the device only sees
dense tables.
"""

import numpy as np

N = 32768
E = 1015808
D = 64
C = 8          # cores
NL = N // C    # 4096 nodes per core
P = 128        # partitions / tile
T = NL // P    # 32 tiles per core
Q = 8          # tiles per chain-quarter
NQ = T // Q
MAXN = 1.0 - 4e-3   # PROJ_EPS boundary for c=1
EPS = 1e-15
MAX_TANH = 15.0
R_STAR = float(np.arctanh(np.float64(1.0 - 4e-3)))  # artanh(MAXN) = 3.10635...


def _build_tables(rows, cols, edge_mask, node_mask):
    """Permute nodes by degree, deal tiles round-robin to cores, and build the
    per-core padded gather tables (int16 indices wrapped the way
    InstDMAGatherAnt wants them) plus matching weight tables."""
    deg = np.bincount(rows, minlength=N)
    order = np.argsort(-deg, kind="stable")
    # global tile j -> core j%C, sorted-slot j//C; slots are then re-ordered so
    # the smallest-K slot runs FIRST (cheap desc-gen before the pipeline
    # fills) and the second-smallest LAST (short exposed tail):
    #   slot order = [smallest, biggest, ..., 2nd smallest]
    S = np.empty(T, dtype=np.int64)
    S[0] = T - 1
    S[1:] = np.arange(T - 1)
    Sinv = np.empty(T, dtype=np.int64)
    Sinv[S] = np.arange(T)
    perm = np.empty(N, dtype=np.int64)
    j = np.arange(N) // P                     # global tile of sorted rank r
    c = j % C
    t = Sinv[j // C]
    p = np.arange(N) % P
    perm[c * NL + t * P + p] = order          # perm[g] = original node id
    pos = np.empty(N, dtype=np.int64)
    pos[perm] = np.arange(N)

    # gather-table row id for permuted position g=(c,t,p):
    #   AllGather concatenates per-core [P, T*D] blocks, so
    #   row_id = c*NL + p*T + t
    gg = np.arange(N)
    gc, gr = gg // NL, gg % NL
    gt, gp_ = gr // P, gr % P
    rowid = gc * NL + gp_ * T + gt            # [g] -> table row
    dstpos = pos[rows]
    eorder = np.argsort(dstpos, kind="stable")
    src_sorted = rowid[pos[cols[eorder]]]     # gather table rows, 0..N-1
    w_sorted = edge_mask[eorder, 0].astype(np.float64)
    cnts = np.bincount(dstpos, minlength=N)
    offs = np.zeros(N + 1, dtype=np.int64)
    np.cumsum(cnts, out=offs[1:])

    # per-slot K: max count over the 8 cores' tiles in that slot
    cnts_g = cnts.reshape(C, T, P)
    Ks = np.maximum(cnts_g.max(axis=(0, 2)), 1).astype(np.int64)   # [T]

    IDXC = int(8 * Ks.sum())
    WTC = int(Ks.sum())
    idx_dev = np.zeros((C, P, IDXC), np.int16)
    wt_dev = np.zeros((C, P, WTC), np.float32)
    nm = node_mask[:, 0].astype(np.float64)
    ioff = woff = 0
    ar = None
    for t in range(T):
        K = int(Ks[t])
        if ar is None or ar.shape[1] != K:
            ar = np.arange(K)[None, :]
        for cc in range(C):
            base = cc * NL + t * P
            cn = cnts[base:base + P]
            take = offs[base:base + P][:, None] + ar          # [P, K]
            valid = ar < cn[:, None]
            take_c = np.minimum(take, E - 1)
            nb = np.where(valid, src_sorted[take_c], 0)
            wl = np.where(valid, w_sorted[take_c], 0.0)
            wl = wl * nm[perm[base:base + P]][:, None]
            il = nb.T.reshape(-1)                             # i = g*128+p
            ch = il.reshape(8 * K, 16).T                      # [16, 8K]
            idx_dev[cc, :, ioff:ioff + 8 * K] = np.tile(ch, (8, 1)).astype(np.int16)
            wt_dev[cc, :, woff:woff + K] = wl.astype(np.float32)
        ioff += 8 * K
        woff += K
    # pad counts per (core, slot, partition) for the pad-subtract path
    pc_dev = np.zeros((C, 1, T * P), np.float32)
    for t in range(T):
        K = int(Ks[t])
        for cc in range(C):
            base = cc * NL + t * P
            pc_dev[cc, 0, t * P:(t + 1) * P] = K - cnts[base:base + P]
    allones = bool(np.all(edge_mask == 1.0) and np.all(node_mask == 1.0))
    return perm, Ks, idx_dev, wt_dev, IDXC, WTC, pc_dev, allones


def _build_program(Ks, IDXC, WTC, use_wt=True, sim=False):
    import concourse.bacc as bacc
    import concourse.bass as bass
    import concourse.mybir as mybir
    import concourse.tile as tile
    from concourse import library_config
    from concourse.masks import make_identity

    f32 = mybir.dt.float32
    i16 = mybir.dt.int16
    AF = mybir.ActivationFunctionType
    OP = mybir.AluOpType
    X = mybir.AxisListType.X

    nc = bacc.Bacc("TRN2", target_bir_lowering=False, debug=False,
                   num_devices=1 if sim else C)

    h_in = nc.dram_tensor("h_in", [P, T * D], f32, kind="ExternalInput")
    idx_in = nc.dram_tensor("idx_in", [P, IDXC], i16, kind="ExternalInput")
    wt_in = nc.dram_tensor("wt_in", [P, WTC], f32, kind="ExternalInput")
    w0t_in = nc.dram_tensor("w0t_in", [2 * D, D], f32, kind="ExternalInput")
    w1t_in = nc.dram_tensor("w1t_in", [2 * D, D], f32, kind="ExternalInput")
    wot_in = nc.dram_tensor("wot_in", [2 * D, 16], f32, kind="ExternalInput")
    pc_in = nc.dram_tensor("pc_in", [1, T * P], f32, kind="ExternalInput")
    out_dram = nc.dram_tensor("out", [P, T * 16], f32, kind="ExternalOutput")
    xt_shard = nc.dram_tensor("xt_shard", [P, T * D], f32)
    xt_table = nc.dram_tensor("xt_table", [N, D], f32, addr_space="Shared")
    groups = [list(range(C))]

    with tile.TileContext(nc) as tc:
        nc.gpsimd.load_library(library_config.mlp)
        import contextlib
        ctx = contextlib.ExitStack()
        with ctx:
            const = ctx.enter_context(tc.tile_pool(name="const", bufs=1))
            sqp = ctx.enter_context(tc.tile_pool(name="sq", bufs=2))
            xtp = ctx.enter_context(tc.tile_pool(name="xtp", bufs=3))
            gp = ctx.enter_context(tc.tile_pool(name="gp", bufs=4))
            scp = ctx.enter_context(tc.tile_pool(name="scp", bufs=3))
            psT = ctx.enter_context(tc.tile_pool(name="psT", bufs=2, space="PSUM"))
            psM = ctx.enter_context(tc.tile_pool(name="psM", bufs=2, space="PSUM"))
            psC = ctx.enter_context(tc.tile_pool(name="psC", bufs=2, space="PSUM"))

            ident = const.tile([P, P], f32)
            make_identity(nc, ident[:])
            # weights duplicated across both partition halves so matmuls can
            # source lhsT from either half of a paired transpose
            w0t_sb = const.tile([2 * D, D], f32)
            w1t_sb = const.tile([2 * D, D], f32)
            wot_sb = const.tile([2 * D, 16], f32)
            idx_sb = const.tile([P, IDXC], i16)
            pc_sb = const.tile([1, T * P], f32)
            if use_wt:
                wt_sb = const.tile([P, WTC], f32)

            u_sb = const.tile([P, T * D], f32)      # raw vector state / agg
            mv_sb = const.tile([P, T * D], f32)     # W@u then scaled messages
            out_sb = const.tile([P, T * 16], f32)
            # long-lived per-node scalars, [P, T] column t = tile t
            A_sb = const.tile([P, T], f32)    # artanh(||x||) of current state
            rn_sb = const.tile([P, T], f32)   # 1 / ||u_raw||
            y_sb = const.tile([P, T], f32)    # A * rn (sigma chain operand)
            t2_sb = const.tile([P, T], f32)   # final tanh argument (layer 2)
            na2_sb = const.tile([P, T], f32)
            nu2_sb = const.tile([P, T], f32)
            mx2_sb = const.tile([P, T], f32)
            sg_sb = const.tile([P, T], f32)   # publish scale sigma per tile
            sf_sb = const.tile([P, T], f32)   # final readout scale

            # load order: h quarter 0 + W0 first (intro/linear need them), the
            # gather metadata afterwards so it streams under the intro compute
            QW = Q * D
            nc.sync.dma_start(out=u_sb[:, 0:QW], in_=h_in[:, 0:QW])
            nc.sync.dma_start(out=w0t_sb[:], in_=w0t_in[:])
            for q in range(1, NQ):
                nc.sync.dma_start(out=u_sb[:, q * QW:(q + 1) * QW],
                                  in_=h_in[:, q * QW:(q + 1) * QW])
            nc.sync.dma_start(out=w1t_sb[:], in_=w1t_in[:])
            nc.sync.dma_start(out=wot_sb[:], in_=wot_in[:])
            nc.sync.dma_start(out=pc_sb[:], in_=pc_in[:])
            nc.sync.dma_start(out=idx_sb[:], in_=idx_in[:])
            if use_wt:
                nc.sync.dma_start(out=wt_sb[:], in_=wt_in[:])

            def ts(t, w=D):
                return slice(t * w, (t + 1) * w)

            def act(out, in_, f, **kw):
                nc.scalar.activation(out, in_, f, **kw)

            def intro_quarter(q):
                """A = min(||h||, R*), rn = 1/||h||, y = A*rn for quarter q."""
                cols = ts(q, Q)
                n02 = scp.tile([P, Q], f32, tag="n02")
                sqb = sqp.tile([P, Q * D], f32, tag="sqb")
                act(sqb[:], u_sb[:, ts(q, Q * D)], AF.Square)
                nc.vector.tensor_reduce(
                    n02[:], sqb[:].rearrange("p (t d) -> p t d", d=D),
                    axis=X, op=OP.add)
                z = scp.tile([P, Q], f32, tag="z0")
                nc.vector.reciprocal(z[:], n02[:])
                act(rn_sb[:, cols], z[:], AF.Sqrt)          # 1/n0
                n0 = scp.tile([P, Q], f32, tag="n0")
                nc.vector.tensor_tensor(n0[:], n02[:], rn_sb[:, cols],
                                        op=OP.mult)         # n0
                nc.vector.tensor_scalar_min(A_sb[:, cols], n0[:], R_STAR)
                nc.vector.tensor_tensor(y_sb[:, cols], A_sb[:, cols],
                                        rn_sb[:, cols], op=OP.mult)

            def mm_tiles(tiles, w_sb, dst_sb, ow):
                """transpose+matmul u[tiles] @ W^T -> dst_sb (free width ow),
                batching up to 4 matmul outputs per PSUM tile/copy."""
                t0, t1 = tiles.start, tiles.stop
                t = t0
                while t < t1:
                    nq = min(4, t1 - t)
                    pm = psM.tile([P, 4 * ow], f32, tag="mv")
                    for j in range(nq):
                        tt = t + j
                        if j % 2 == 0:
                            np2 = min(2, t1 - tt)
                            ps = psT.tile([P, P], f32, tag="xT2")
                            nc.tensor.transpose(
                                out=ps[0:np2 * D, :],
                                in_=u_sb[:, tt * D:(tt + np2) * D],
                                identity=ident[:])
                            xT2 = xtp.tile([P, P], f32, tag="xT2sb")
                            nc.vector.tensor_copy(xT2[0:np2 * D, :],
                                                  ps[0:np2 * D, :])
                        h2 = j % 2
                        nc.tensor.matmul(out=pm[:, j * ow:(j + 1) * ow],
                                         lhsT=xT2[h2 * D:(h2 + 1) * D, :],
                                         rhs=w_sb[h2 * D:(h2 + 1) * D, :],
                                         start=True, stop=True)
                    act(dst_sb[:, t * ow:(t + nq) * ow], pm[:, :nq * ow],
                        AF.Copy)
                    t += nq

            def linear_block(w_sb, tiles, publish=True):
                """mv_raw = u @ W^T for `tiles`; sigma = min(y, R*/mraw);
                mv *= sigma; publish.  (y = A*rn precomputed per node.)"""
                t0, t1 = tiles.start, tiles.stop
                nt = t1 - t0
                mm_tiles(tiles, w_sb, mv_sb, D)
                # batched ||mv||^2 then the fused sigma chain
                for j in range((nt + Q - 1) // Q):
                    c0 = t0 + j * Q
                    c1 = min(c0 + Q, t1)
                    sqb = sqp.tile([P, Q * D], f32, tag="sqb")
                    act(sqb[:, :(c1 - c0) * D], mv_sb[:, c0 * D:c1 * D], AF.Square)
                    nc.vector.tensor_reduce(
                        mx2_sb[:, c0:c1],
                        sqb[:, :(c1 - c0) * D].rearrange("p (t d) -> p t d", d=D),
                        axis=X, op=OP.add)
                cols = slice(t0, t1)
                rm = scp.tile([P, T], f32, tag="rm")
                nc.vector.reciprocal(rm[:, cols], mx2_sb[:, cols])
                act(rm[:, cols], rm[:, cols], AF.Sqrt, scale=R_STAR * R_STAR)
                nc.vector.tensor_tensor(sg_sb[:, cols], y_sb[:, cols],
                                        rm[:, cols], op=OP.min)
                # mv *= sigma, broadcast over d in one strided multiply
                mv3 = mv_sb[:, t0 * D:t1 * D].rearrange("p (t d) -> p t d", d=D)
                sg_ap = sg_sb[:, cols]
                sgv = bass.AP(sg_ap.tensor, sg_ap.offset,
                              list(sg_ap.ap) + [[0, D]])
                nc.vector.tensor_tensor(mv3, mv3, sgv, op=OP.mult)
                if publish:
                    nc.sync.dma_start(out=xt_shard[:, t0 * D:t1 * D],
                                      in_=mv_sb[:, t0 * D:t1 * D])

            def readout_block(tiles):
                mm_tiles(tiles, wot_sb, out_sb, 16)

            def allgather():
                if sim:
                    nc.sync.dma_start(
                        out=xt_table[0:NL, :].rearrange("(p x) d -> p x d", p=P),
                        in_=xt_shard[:].rearrange("p (x d) -> p x d", d=D))
                else:
                    nc.gpsimd.collective_compute(
                        "AllGather", mybir.AluOpType.bypass, replica_groups=groups,
                        ins=[xt_shard[:, :]], outs=[xt_table[:, :]])

            def post_agg_group(g0, g1, last_layer):
                """norms of agg & relu(agg); s' chain; t = nu*s'; A'/rn' (or
                stash t2 for the final tanh)."""
                w = g1 - g0
                cols = slice(g0, g1)
                csl = slice(g0 * D, g1 * D)
                sqb = sqp.tile([P, Q * D], f32, tag="sqb")
                act(sqb[:, :w * D], u_sb[:, csl], AF.Square)
                nc.vector.tensor_reduce(
                    na2_sb[:, cols],
                    sqb[:, :w * D].rearrange("p (t d) -> p t d", d=D),
                    axis=X, op=OP.add)
                act(u_sb[:, csl], u_sb[:, csl], AF.Relu)
                sqb2 = sqp.tile([P, Q * D], f32, tag="sqb")
                act(sqb2[:, :w * D], u_sb[:, csl], AF.Square)
                nc.vector.tensor_reduce(
                    nu2_sb[:, cols],
                    sqb2[:, :w * D].rearrange("p (t d) -> p t d", d=D),
                    axis=X, op=OP.add)
                # s' = min(1, R*/na);  rn = 1/nu;  t = nu*s';  A' = min(t, R*)
                sp = scp.tile([P, Q], f32, tag="sp")
                nc.vector.reciprocal(sp[:, :w], na2_sb[:, cols])
                act(sp[:, :w], sp[:, :w], AF.Sqrt, scale=R_STAR * R_STAR)
                nc.vector.tensor_scalar_min(sp[:, :w], sp[:, :w], 1.0)
                z2 = scp.tile([P, Q], f32, tag="z2")
                nc.vector.tensor_scalar_max(z2[:, :w], nu2_sb[:, cols], 1e-30)
                nc.vector.reciprocal(z2[:, :w], z2[:, :w])
                act(rn_sb[:, cols], z2[:, :w], AF.Sqrt)
                nu = scp.tile([P, Q], f32, tag="nu")
                nc.vector.tensor_tensor(nu[:, :w], nu2_sb[:, cols],
                                        rn_sb[:, cols], op=OP.mult)
                tq = scp.tile([P, Q], f32, tag="tq")
                nc.vector.tensor_tensor(tq[:, :w], nu[:, :w], sp[:, :w],
                                        op=OP.mult)
                if last_layer:
                    nc.vector.tensor_scalar_min(t2_sb[:, cols], tq[:, :w],
                                                MAX_TANH)
                else:
                    nc.vector.tensor_scalar_min(A_sb[:, cols], tq[:, :w],
                                                R_STAR)
                    nc.vector.tensor_tensor(y_sb[:, cols], A_sb[:, cols],
                                            rn_sb[:, cols], op=OP.mult)

            # chain-group boundaries: big groups early (hidden under DMA),
            # small groups at the tail to shrink the exposed serial end
            GROUPS = [(0, 8), (8, 16), (16, 24), (24, 28), (28, 30),
                      (30, 31), (31, 32)]
            def gather_layer(layer):
                """aggregate messages; per finished chain-group run the
                post-agg chain and the next stage's linear work."""
                row0_sb = scp.tile([1, D], f32, tag="row0")
                ioff = woff = 0
                gidx = 0
                for t in range(T):
                    K = int(Ks[t])
                    g = gp.tile([P, K * D], f32, tag="G")
                    g3 = g[:].rearrange("p (k d) -> p k d", d=D)
                    nc.gpsimd.dma_gather(
                        g3, xt_table[:, :], idx_sb[:, ioff:ioff + 8 * K],
                        num_idxs=P * K, num_idxs_reg=P * K, elem_size=D,
                        single_packet=False)
                    if t == 0 and not use_wt:
                        # row0 value for the pad-subtract; queued behind the
                        # first gather so it doesn't delay the pipeline start
                        nc.sync.dma_start(out=row0_sb[:], in_=xt_table[0:1, :])
                    if use_wt:
                        wt_ap = wt_sb[:, woff:woff + K]
                        wv = bass.AP(wt_ap.tensor, wt_ap.offset,
                                     list(wt_ap.ap) + [[0, D]])
                        nc.vector.tensor_tensor(g3, g3, wv, op=OP.mult)
                    nc.vector.tensor_reduce(
                        u_sb[:, ts(t)],
                        g[:].rearrange("p (k d) -> p d k", d=D),
                        axis=X, op=OP.add)
                    if not use_wt:
                        corr_ps = psC.tile([P, D], f32, tag="corr")
                        nc.tensor.matmul(
                            out=corr_ps[:], lhsT=pc_sb[0:1, t * P:(t + 1) * P],
                            rhs=row0_sb[0:1, :], start=True, stop=True)
                        nc.vector.tensor_tensor(u_sb[:, ts(t)], u_sb[:, ts(t)],
                                                corr_ps[:], op=OP.subtract)
                    ioff += 8 * K
                    woff += K
                    if t + 1 == GROUPS[gidx][1]:
                        g0, g1 = GROUPS[gidx]
                        gidx += 1
                        post_agg_group(g0, g1, last_layer=(layer == 1))
                        if layer == 0:
                            linear_block(w1t_sb, slice(g0, g1))
                        else:
                            readout_block(slice(g0, g1))

            # ---- layer 1 linear + publish -----------------------------------
            for q in range(NQ):
                intro_quarter(q)
                linear_block(w0t_sb, slice(q * Q, (q + 1) * Q))
            allgather()
            gather_layer(0)
            allgather()
            gather_layer(1)

            # ---- tail: single Tanh, final scale broadcast, store ------------
            th = scp.tile([P, T], f32, tag="th")
            act(th[:], t2_sb[:], AF.Tanh)
            nc.vector.tensor_scalar_min(th[:], th[:], MAXN)
            nc.vector.tensor_tensor(sf_sb[:], th[:], rn_sb[:], op=OP.mult)
            o3 = out_sb[:].rearrange("p (t j) -> p t j", j=16)
            sf_ap = sf_sb[:]
            sfv = bass.AP(sf_ap.tensor, sf_ap.offset, list(sf_ap.ap) + [[0, 16]])
            nc.vector.tensor_tensor(o3, o3, sfv, op=OP.mult)
            nc.sync.dma_start(out=out_dram[:], in_=out_sb[:])
    nc.compile()
    return nc


def kernel(h, distances, rows, cols, node_mask, edge_mask,
           W0, b0, W1, b1, W_out, b_out, _trace=False):
    from concourse.bass_utils import run_bass_kernel_spmd

    h = np.asarray(h, dtype=np.float32)
    rows = np.asarray(rows).astype(np.int64)
    cols = np.asarray(cols).astype(np.int64)
    node_mask = np.asarray(node_mask, dtype=np.float32)
    edge_mask = np.asarray(edge_mask, dtype=np.float32)
    assert not np.any(np.asarray(b0)) and not np.any(np.asarray(b1)) and \
        not np.any(np.asarray(b_out)), "nonzero biases unsupported"

    perm, Ks, idx_dev, wt_dev, IDXC, WTC, pc_dev, allones = _build_tables(
        rows, cols, edge_mask, node_mask)

    hp = h[perm].reshape(C, T, P, D).transpose(0, 2, 1, 3).reshape(C, P, T * D)
    w0t = np.ascontiguousarray(np.vstack([np.asarray(W0, np.float32).T] * 2))
    w1t = np.ascontiguousarray(np.vstack([np.asarray(W1, np.float32).T] * 2))
    wot = np.ascontiguousarray(np.vstack([np.asarray(W_out, np.float32).T] * 2))

    nc = _build_program(Ks, IDXC, WTC, use_wt=not allones)
    in_maps = [{
        "h_in": np.ascontiguousarray(hp[c]),
        "idx_in": idx_dev[c],
        "wt_in": wt_dev[c],
        "w0t_in": w0t, "w1t_in": w1t, "wot_in": wot,
        "pc_in": pc_dev[c],
    } for c in range(C)]
    res = run_bass_kernel_spmd(nc, in_maps, list(range(C)), trace=_trace)
    od = np.stack([res.results[c]["out"] for c in range(C)])
    od = od.reshape(C, P, T, 16).transpose(0, 2, 1, 3).reshape(N, 16)
    out = np.empty((N, 16), np.float32)
    out[perm] = od
    if _trace:
        return out, res
    return out


# revision 44
# speedup vs baseline: 1.3873x; 1.0006x over previous
"""HGCN decoder on 8 trn2 NeuronCores.

Strategy: nodes are sorted by in-degree, grouped into 128-node tiles, and the
tiles are dealt round-robin across the 8 cores (graph-parallel by destination
node).  Each core:
  - runs the node-wise hyperbolic math on its 4096 nodes,
  - publishes its tangent-space table shard, AllGathers the full [32768, 64]
    table to DRAM,
  - aggregates messages with `dma_gather` (padded per-tile CSR) followed by a
    strided reduce on the vector engine,
  - finishes with the euclidean readout matmul.

Key algebraic simplifications vs the literal reference (all exact up to fp
rounding, validated against the jax reference):
  - proj-then-logmap0 collapses: artanh(min(tanh(r), 1-eps)) == min(r, R*)
    with R* = artanh(1-eps).  This removes every Ln/artanh evaluation and all
    but one final Tanh, so the scalar chains are pure min/mult/recip/sqrt.
  - positive per-node scales commute through relu and matmuls, so the vector
    state is kept UNSCALED (raw) and only combined per-node scalars are
    carried between stages.  The only materialized scalings are the published
    message table (xt = mv_raw * sigma) and the final readout.
Activation-table note: the whole kernel needs only {Square, Sqrt, Relu, Copy}
(one table) plus a single trailing Tanh (second table) => 2 table loads total.

All graph preprocessing happens host-side in numpy; the device only sees
dense tables.
"""

import numpy as np

N = 32768
E = 1015808
D = 64
C = 8          # cores
NL = N // C    # 4096 nodes per core
P = 128        # partitions / tile
T = NL // P    # 32 tiles per core
Q = 8          # tiles per chain-quarter
NQ = T // Q
MAXN = 1.0 - 4e-3   # PROJ_EPS boundary for c=1
EPS = 1e-15
MAX_TANH = 15.0
R_STAR = float(np.arctanh(np.float64(1.0 - 4e-3)))  # artanh(MAXN) = 3.10635...


def _build_tables(rows, cols, edge_mask, node_mask):
    """Permute nodes by degree, deal tiles round-robin to cores, and build the
    per-core padded gather tables (int16 indices wrapped the way
    InstDMAGatherAnt wants them) plus matching weight tables."""
    deg = np.bincount(rows, minlength=N)
    order = np.argsort(-deg, kind="stable")
    # global tile j -> core j%C, sorted-slot j//C; slots are then re-ordered so
    # the smallest-K slot runs FIRST (cheap desc-gen before the pipeline
    # fills) and the second-smallest LAST (short exposed tail):
    #   slot order = [smallest, biggest, ..., 2nd smallest]
    S = np.empty(T, dtype=np.int64)
    S[0] = T - 1
    S[1:] = np.arange(T - 1)
    Sinv = np.empty(T, dtype=np.int64)
    Sinv[S] = np.arange(T)
    perm = np.empty(N, dtype=np.int64)
    j = np.arange(N) // P                     # global tile of sorted rank r
    c = j % C
    t = Sinv[j // C]
    p = np.arange(N) % P
    perm[c * NL + t * P + p] = order          # perm[g] = original node id
    pos = np.empty(N, dtype=np.int64)
    pos[perm] = np.arange(N)

    # gather-table row id for permuted position g=(c,t,p):
    #   AllGather concatenates per-core [P, T*D] blocks, so
    #   row_id = c*NL + p*T + t
    gg = np.arange(N)
    gc, gr = gg // NL, gg % NL
    gt, gp_ = gr // P, gr % P
    rowid = gc * NL + gp_ * T + gt            # [g] -> table row
    dstpos = pos[rows]
    eorder = np.argsort(dstpos, kind="stable")
    src_sorted = rowid[pos[cols[eorder]]]     # gather table rows, 0..N-1
    w_sorted = edge_mask[eorder, 0].astype(np.float64)
    cnts = np.bincount(dstpos, minlength=N)
    offs = np.zeros(N + 1, dtype=np.int64)
    np.cumsum(cnts, out=offs[1:])

    # per-slot K: max count over the 8 cores' tiles in that slot
    cnts_g = cnts.reshape(C, T, P)
    Ks = np.maximum(cnts_g.max(axis=(0, 2)), 1).astype(np.int64)   # [T]

    IDXC = int(8 * Ks.sum())
    WTC = int(Ks.sum())
    idx_dev = np.zeros((C, P, IDXC), np.int16)
    wt_dev = np.zeros((C, P, WTC), np.float32)
    nm = node_mask[:, 0].astype(np.float64)
    ioff = woff = 0
    ar = None
    for t in range(T):
        K = int(Ks[t])
        if ar is None or ar.shape[1] != K:
            ar = np.arange(K)[None, :]
        for cc in range(C):
            base = cc * NL + t * P
            cn = cnts[base:base + P]
            take = offs[base:base + P][:, None] + ar          # [P, K]
            valid = ar < cn[:, None]
            take_c = np.minimum(take, E - 1)
            nb = np.where(valid, src_sorted[take_c], 0)
            wl = np.where(valid, w_sorted[take_c], 0.0)
            wl = wl * nm[perm[base:base + P]][:, None]
            il = nb.T.reshape(-1)                             # i = g*128+p
            ch = il.reshape(8 * K, 16).T                      # [16, 8K]
            idx_dev[cc, :, ioff:ioff + 8 * K] = np.tile(ch, (8, 1)).astype(np.int16)
            wt_dev[cc, :, woff:woff + K] = wl.astype(np.float32)
        ioff += 8 * K
        woff += K
    # pad counts per (core, slot, partition) for the pad-subtract path
    pc_dev = np.zeros((C, 1, T * P), np.float32)
    for t in range(T):
        K = int(Ks[t])
        for cc in range(C):
            base = cc * NL + t * P
            pc_dev[cc, 0, t * P:(t + 1) * P] = K - cnts[base:base + P]
    allones = bool(np.all(edge_mask == 1.0) and np.all(node_mask == 1.0))
    return perm, Ks, idx_dev, wt_dev, IDXC, WTC, pc_dev, allones


def _build_program(Ks, IDXC, WTC, use_wt=True, sim=False):
    import concourse.bacc as bacc
    import concourse.bass as bass
    import concourse.mybir as mybir
    import concourse.tile as tile
    from concourse import library_config
    from concourse.masks import make_identity

    f32 = mybir.dt.float32
    i16 = mybir.dt.int16
    AF = mybir.ActivationFunctionType
    OP = mybir.AluOpType
    X = mybir.AxisListType.X

    nc = bacc.Bacc("TRN2", target_bir_lowering=False, debug=False,
                   num_devices=1 if sim else C)

    h_in = nc.dram_tensor("h_in", [P, T * D], f32, kind="ExternalInput")
    idx_in = nc.dram_tensor("idx_in", [P, IDXC], i16, kind="ExternalInput")
    wt_in = nc.dram_tensor("wt_in", [P, WTC], f32, kind="ExternalInput")
    w0t_in = nc.dram_tensor("w0t_in", [2 * D, D], f32, kind="ExternalInput")
    w1t_in = nc.dram_tensor("w1t_in", [2 * D, D], f32, kind="ExternalInput")
    wot_in = nc.dram_tensor("wot_in", [2 * D, 16], f32, kind="ExternalInput")
    pc_in = nc.dram_tensor("pc_in", [1, T * P], f32, kind="ExternalInput")
    out_dram = nc.dram_tensor("out", [P, T * 16], f32, kind="ExternalOutput")
    xt_shard = nc.dram_tensor("xt_shard", [P, T * D], f32)
    xt_table = nc.dram_tensor("xt_table", [N, D], f32, addr_space="Shared")
    groups = [list(range(C))]

    with tile.TileContext(nc) as tc:
        nc.gpsimd.load_library(library_config.mlp)
        import contextlib
        ctx = contextlib.ExitStack()
        with ctx:
            const = ctx.enter_context(tc.tile_pool(name="const", bufs=1))
            sqp = ctx.enter_context(tc.tile_pool(name="sq", bufs=2))
            xtp = ctx.enter_context(tc.tile_pool(name="xtp", bufs=3))
            gp = ctx.enter_context(tc.tile_pool(name="gp", bufs=4))
            scp = ctx.enter_context(tc.tile_pool(name="scp", bufs=3))
            psT = ctx.enter_context(tc.tile_pool(name="psT", bufs=2, space="PSUM"))
            psM = ctx.enter_context(tc.tile_pool(name="psM", bufs=2, space="PSUM"))
            psC = ctx.enter_context(tc.tile_pool(name="psC", bufs=2, space="PSUM"))

            ident = const.tile([P, P], f32)
            make_identity(nc, ident[:])
            # weights duplicated across both partition halves so matmuls can
            # source lhsT from either half of a paired transpose
            w0t_sb = const.tile([2 * D, D], f32)
            w1t_sb = const.tile([2 * D, D], f32)
            wot_sb = const.tile([2 * D, 16], f32)
            idx_sb = const.tile([P, IDXC], i16)
            pc_sb = const.tile([1, T * P], f32)
            if use_wt:
                wt_sb = const.tile([P, WTC], f32)

            u_sb = const.tile([P, T * D], f32)      # raw vector state / agg
            mv_sb = const.tile([P, T * D], f32)     # W@u then scaled messages
            out_sb = const.tile([P, T * 16], f32)
            # long-lived per-node scalars, [P, T] column t = tile t
            A_sb = const.tile([P, T], f32)    # artanh(||x||) of current state
            rn_sb = const.tile([P, T], f32)   # 1 / ||u_raw||
            y_sb = const.tile([P, T], f32)    # A * rn (sigma chain operand)
            t2_sb = const.tile([P, T], f32)   # final tanh argument (layer 2)
            na2_sb = const.tile([P, T], f32)
            nu2_sb = const.tile([P, T], f32)
            mx2_sb = const.tile([P, T], f32)
            sg_sb = const.tile([P, T], f32)   # publish scale sigma per tile
            sf_sb = const.tile([P, T], f32)   # final readout scale

            # load order: h quarter 0 + W0 first (intro/linear need them), the
            # gather metadata afterwards so it streams under the intro compute
            QW = Q * D
            nc.sync.dma_start(out=u_sb[:, 0:QW], in_=h_in[:, 0:QW])
            nc.sync.dma_start(out=w0t_sb[:], in_=w0t_in[:])
            for q in range(1, NQ):
                nc.sync.dma_start(out=u_sb[:, q * QW:(q + 1) * QW],
                                  in_=h_in[:, q * QW:(q + 1) * QW])
            nc.sync.dma_start(out=w1t_sb[:], in_=w1t_in[:])
            nc.sync.dma_start(out=wot_sb[:], in_=wot_in[:])
            nc.sync.dma_start(out=pc_sb[:], in_=pc_in[:])
            nc.sync.dma_start(out=idx_sb[:], in_=idx_in[:])
            if use_wt:
                nc.sync.dma_start(out=wt_sb[:], in_=wt_in[:])

            def ts(t, w=D):
                return slice(t * w, (t + 1) * w)

            def act(out, in_, f, **kw):
                nc.scalar.activation(out, in_, f, **kw)

            def intro_quarter(q):
                """A = min(||h||, R*), rn = 1/||h||, y = A*rn for quarter q."""
                cols = ts(q, Q)
                n02 = scp.tile([P, Q], f32, tag="n02")
                sqb = sqp.tile([P, Q * D], f32, tag="sqb")
                act(sqb[:], u_sb[:, ts(q, Q * D)], AF.Square)
                nc.vector.tensor_reduce(
                    n02[:], sqb[:].rearrange("p (t d) -> p t d", d=D),
                    axis=X, op=OP.add)
                z = scp.tile([P, Q], f32, tag="z0")
                nc.vector.reciprocal(z[:], n02[:])
                act(rn_sb[:, cols], z[:], AF.Sqrt)          # 1/n0
                n0 = scp.tile([P, Q], f32, tag="n0")
                nc.vector.tensor_tensor(n0[:], n02[:], rn_sb[:, cols],
                                        op=OP.mult)         # n0
                nc.vector.tensor_scalar_min(A_sb[:, cols], n0[:], R_STAR)
                nc.vector.tensor_tensor(y_sb[:, cols], A_sb[:, cols],
                                        rn_sb[:, cols], op=OP.mult)

            def mm_tiles(tiles, w_sb, dst_sb, ow):
                """transpose+matmul u[tiles] @ W^T -> dst_sb (free width ow),
                batching up to 4 matmul outputs per PSUM tile/copy."""
                t0, t1 = tiles.start, tiles.stop
                t = t0
                while t < t1:
                    nq = min(4, t1 - t)
                    pm = psM.tile([P, 4 * ow], f32, tag="mv")
                    for j in range(nq):
                        tt = t + j
                        if j % 2 == 0:
                            np2 = min(2, t1 - tt)
                            ps = psT.tile([P, P], f32, tag="xT2")
                            nc.tensor.transpose(
                                out=ps[0:np2 * D, :],
                                in_=u_sb[:, tt * D:(tt + np2) * D],
                                identity=ident[:])
                            xT2 = xtp.tile([P, P], f32, tag="xT2sb")
                            nc.vector.tensor_copy(xT2[0:np2 * D, :],
                                                  ps[0:np2 * D, :])
                        h2 = j % 2
                        nc.tensor.matmul(out=pm[:, j * ow:(j + 1) * ow],
                                         lhsT=xT2[h2 * D:(h2 + 1) * D, :],
                                         rhs=w_sb[h2 * D:(h2 + 1) * D, :],
                                         start=True, stop=True)
                    act(dst_sb[:, t * ow:(t + nq) * ow], pm[:, :nq * ow],
                        AF.Copy)
                    t += nq

            def linear_block(w_sb, tiles, publish=True):
                """mv_raw = u @ W^T for `tiles`; sigma = min(y, R*/mraw);
                mv *= sigma; publish.  (y = A*rn precomputed per node.)"""
                t0, t1 = tiles.start, tiles.stop
                nt = t1 - t0
                mm_tiles(tiles, w_sb, mv_sb, D)
                # batched ||mv||^2 then the fused sigma chain
                for j in range((nt + Q - 1) // Q):
                    c0 = t0 + j * Q
                    c1 = min(c0 + Q, t1)
                    sqb = sqp.tile([P, Q * D], f32, tag="sqb")
                    act(sqb[:, :(c1 - c0) * D], mv_sb[:, c0 * D:c1 * D], AF.Square)
                    nc.vector.tensor_reduce(
                        mx2_sb[:, c0:c1],
                        sqb[:, :(c1 - c0) * D].rearrange("p (t d) -> p t d", d=D),
                        axis=X, op=OP.add)
                cols = slice(t0, t1)
                rm = scp.tile([P, T], f32, tag="rm")
                nc.vector.reciprocal(rm[:, cols], mx2_sb[:, cols])
                act(rm[:, cols], rm[:, cols], AF.Sqrt, scale=R_STAR * R_STAR)
                nc.vector.tensor_tensor(sg_sb[:, cols], y_sb[:, cols],
                                        rm[:, cols], op=OP.min)
                # mv *= sigma, broadcast over d in one strided multiply
                mv3 = mv_sb[:, t0 * D:t1 * D].rearrange("p (t d) -> p t d", d=D)
                sg_ap = sg_sb[:, cols]
                sgv = bass.AP(sg_ap.tensor, sg_ap.offset,
                              list(sg_ap.ap) + [[0, D]])
                nc.vector.tensor_tensor(mv3, mv3, sgv, op=OP.mult)
                if publish:
                    nc.sync.dma_start(out=xt_shard[:, t0 * D:t1 * D],
                                      in_=mv_sb[:, t0 * D:t1 * D])

            def readout_block(tiles):
                mm_tiles(tiles, wot_sb, out_sb, 16)

            def allgather():
                if sim:
                    nc.sync.dma_start(
                        out=xt_table[0:NL, :].rearrange("(p x) d -> p x d", p=P),
                        in_=xt_shard[:].rearrange("p (x d) -> p x d", d=D))
                else:
                    nc.gpsimd.collective_compute(
                        "AllGather", mybir.AluOpType.bypass, replica_groups=groups,
                        ins=[xt_shard[:, :]], outs=[xt_table[:, :]])

            def post_agg_group(g0, g1, last_layer):
                """norms of agg & relu(agg); s' chain; t = nu*s'; A'/rn' (or
                stash t2 for the final tanh)."""
                w = g1 - g0
                cols = slice(g0, g1)
                csl = slice(g0 * D, g1 * D)
                sqb = sqp.tile([P, Q * D], f32, tag="sqb")
                act(sqb[:, :w * D], u_sb[:, csl], AF.Square)
                nc.vector.tensor_reduce(
                    na2_sb[:, cols],
                    sqb[:, :w * D].rearrange("p (t d) -> p t d", d=D),
                    axis=X, op=OP.add)
                act(u_sb[:, csl], u_sb[:, csl], AF.Relu)
                sqb2 = sqp.tile([P, Q * D], f32, tag="sqb")
                act(sqb2[:, :w * D], u_sb[:, csl], AF.Square)
                nc.vector.tensor_reduce(
                    nu2_sb[:, cols],
                    sqb2[:, :w * D].rearrange("p (t d) -> p t d", d=D),
                    axis=X, op=OP.add)
                # s' = min(1, R*/na);  rn = 1/nu;  t = nu*s';  A' = min(t, R*)
                sp = scp.tile([P, Q], f32, tag="sp")
                nc.vector.reciprocal(sp[:, :w], na2_sb[:, cols])
                act(sp[:, :w], sp[:, :w], AF.Sqrt, scale=R_STAR * R_STAR)
                nc.vector.tensor_scalar_min(sp[:, :w], sp[:, :w], 1.0)
                z2 = scp.tile([P, Q], f32, tag="z2")
                nc.vector.tensor_scalar_max(z2[:, :w], nu2_sb[:, cols], 1e-30)
                nc.vector.reciprocal(z2[:, :w], z2[:, :w])
                act(rn_sb[:, cols], z2[:, :w], AF.Sqrt)
                nu = scp.tile([P, Q], f32, tag="nu")
                nc.vector.tensor_tensor(nu[:, :w], nu2_sb[:, cols],
                                        rn_sb[:, cols], op=OP.mult)
                tq = scp.tile([P, Q], f32, tag="tq")
                nc.vector.tensor_tensor(tq[:, :w], nu[:, :w], sp[:, :w],
                                        op=OP.mult)
                if last_layer:
                    nc.vector.tensor_scalar_min(t2_sb[:, cols], tq[:, :w],
                                                MAX_TANH)
                else:
                    nc.vector.tensor_scalar_min(A_sb[:, cols], tq[:, :w],
                                                R_STAR)
                    nc.vector.tensor_tensor(y_sb[:, cols], A_sb[:, cols],
                                            rn_sb[:, cols], op=OP.mult)

            # chain-group boundaries: big groups early (hidden under DMA),
            # small groups at the tail to shrink the exposed serial end
            GROUPS = [(0, 8), (8, 16), (16, 24), (24, 28), (28, 30),
                      (30, 31), (31, 32)]
            def gather_layer(layer):
                """aggregate messages; per finished chain-group run the
                post-agg chain and the next stage's linear work."""
                row0_sb = scp.tile([1, D], f32, tag="row0")
                ioff = woff = 0
                gidx = 0
                for t in range(T):
                    K = int(Ks[t])
                    g = gp.tile([P, K * D], f32, tag="G")
                    g3 = g[:].rearrange("p (k d) -> p k d", d=D)
                    nc.gpsimd.dma_gather(
                        g3, xt_table[:, :], idx_sb[:, ioff:ioff + 8 * K],
                        num_idxs=P * K, num_idxs_reg=P * K, elem_size=D,
                        single_packet=False)
                    if t == 0 and not use_wt:
                        # row0 value for the pad-subtract; queued behind the
                        # first gather so it doesn't delay the pipeline start
                        nc.sync.dma_start(out=row0_sb[:], in_=xt_table[0:1, :])
                    if use_wt:
                        wt_ap = wt_sb[:, woff:woff + K]
                        wv = bass.AP(wt_ap.tensor, wt_ap.offset,
                                     list(wt_ap.ap) + [[0, D]])
                        nc.vector.tensor_tensor(g3, g3, wv, op=OP.mult)
                    nc.vector.tensor_reduce(
                        u_sb[:, ts(t)],
                        g[:].rearrange("p (k d) -> p d k", d=D),
                        axis=X, op=OP.add)
                    if not use_wt:
                        corr_ps = psC.tile([P, D], f32, tag="corr")
                        nc.tensor.matmul(
                            out=corr_ps[:], lhsT=pc_sb[0:1, t * P:(t + 1) * P],
                            rhs=row0_sb[0:1, :], start=True, stop=True)
                        nc.vector.tensor_tensor(u_sb[:, ts(t)], u_sb[:, ts(t)],
                                                corr_ps[:], op=OP.subtract)
                    ioff += 8 * K
                    woff += K
                    if t + 1 == GROUPS[gidx][1]:
                        g0, g1 = GROUPS[gidx]
                        gidx += 1
                        post_agg_group(g0, g1, last_layer=(layer == 1))
                        if layer == 0:
                            linear_block(w1t_sb, slice(g0, g1))
                        else:
                            readout_block(slice(g0, g1))

            # ---- layer 1 linear + publish -----------------------------------
            for h in range(2):
                intro_quarter(2 * h)
                intro_quarter(2 * h + 1)
                linear_block(w0t_sb, slice(16 * h, 16 * (h + 1)))
            allgather()
            gather_layer(0)
            allgather()
            gather_layer(1)

            # ---- tail: single Tanh, final scale broadcast, store ------------
            th = scp.tile([P, T], f32, tag="th")
            act(th[:], t2_sb[:], AF.Tanh)
            nc.vector.tensor_scalar_min(th[:], th[:], MAXN)
            nc.vector.tensor_tensor(sf_sb[:], th[:], rn_sb[:], op=OP.mult)
            o3 = out_sb[:].rearrange("p (t j) -> p t j", j=16)
            sf_ap = sf_sb[:]
            sfv = bass.AP(sf_ap.tensor, sf_ap.offset, list(sf_ap.ap) + [[0, 16]])
            nc.vector.tensor_tensor(o3, o3, sfv, op=OP.mult)
            nc.sync.dma_start(out=out_dram[:], in_=out_sb[:])
    nc.compile()
    return nc


def kernel(h, distances, rows, cols, node_mask, edge_mask,
           W0, b0, W1, b1, W_out, b_out, _trace=False):
    from concourse.bass_utils import run_bass_kernel_spmd

    h = np.asarray(h, dtype=np.float32)
    rows = np.asarray(rows).astype(np.int64)
    cols = np.asarray(cols).astype(np.int64)
    node_mask = np.asarray(node_mask, dtype=np.float32)
    edge_mask = np.asarray(edge_mask, dtype=np.float32)
    assert not np.any(np.asarray(b0)) and not np.any(np.asarray(b1)) and \
        not np.any(np.asarray(b_out)), "nonzero biases unsupported"

    perm, Ks, idx_dev, wt_dev, IDXC, WTC, pc_dev, allones = _build_tables(
        rows, cols, edge_mask, node_mask)

    hp = h[perm].reshape(C, T, P, D).transpose(0, 2, 1, 3).reshape(C, P, T * D)
    w0t = np.ascontiguousarray(np.vstack([np.asarray(W0, np.float32).T] * 2))
    w1t = np.ascontiguousarray(np.vstack([np.asarray(W1, np.float32).T] * 2))
    wot = np.ascontiguousarray(np.vstack([np.asarray(W_out, np.float32).T] * 2))

    nc = _build_program(Ks, IDXC, WTC, use_wt=not allones)
    in_maps = [{
        "h_in": np.ascontiguousarray(hp[c]),
        "idx_in": idx_dev[c],
        "wt_in": wt_dev[c],
        "w0t_in": w0t, "w1t_in": w1t, "wot_in": wot,
        "pc_in": pc_dev[c],
    } for c in range(C)]
    res = run_bass_kernel_spmd(nc, in_maps, list(range(C)), trace=_trace)
    od = np.stack([res.results[c]["out"] for c in range(C)])
    od = od.reshape(C, P, T, 16).transpose(0, 2, 1, 3).reshape(N, 16)
    out = np.empty((N, 16), np.float32)
    out[perm] = od
    if _trace:
        return out, res
    return out
